# revision 23
# baseline (speedup 1.0000x reference)
"""Deformable Transformer Encoder Layer — Trainium2 Bass kernel (v2).

Sharding: 8 cores = (batch b in 0..3) x (query-half h in 0..1).
Each core computes the full layer for its (b, query-half) slice.

Sampling strategy (per core):
  - value projection over the FULL batch image (PE), stored bf16.
  - zero-PADDED per-level grid v1p ((H+2)x(W+2) rows per level) makes
    grid_sample zero-padding semantics exact with plain clip+floor math
    (no OOB masks).
  - K=4 x-pre-lerped grids G_s = (1-s/4) v1p + (s/4) shift_x(v1p),
    laid out [s][m][row][32ch] bf16, so one sample point needs a single
    contiguous 128B run (y-row pair) fetched via gpsimd indirect DMA.
    x-fraction is quantized to 1/4 pixel (error ~1e-3 of value scale).
  - blend: per head, multiply gathered pairs by combined
    attention*(1-wy, wy) weights (Pool engine) and a segmented reduce
    (DVE), followed by out-proj / LN / FFN / LN as in the baseline.
No cross-core communication; host reassembles the output.
"""

import os
import sys
import numpy as np

for _p in ("/opt/trn_rl_repo", "/root/.axon_site/_ro/trn_rl_repo"):
    if os.path.isdir(_p) and _p not in sys.path:
        sys.path.insert(0, _p)

import concourse.bass as bass
import concourse.mybir as mybir
import concourse.tile as tile
from concourse import bacc
from concourse.bass import AP

F32 = mybir.dt.float32
BF16 = mybir.dt.bfloat16
I32 = mybir.dt.int32
I16 = mybir.dt.int16
AF = mybir.ActivationFunctionType
OP = mybir.AluOpType
AX = mybir.AxisListType

# Problem constants (checked against inputs at runtime on host).
M, LV, P, DM, DH, DF = 8, 4, 4, 256, 32, 1024
L = 11253
B = 4
LC = 5627           # queries per core (split [5627, 5626])
LCPAD = 5632        # 44 * 128
NBLK = LCPAD // 128
EPS = 1e-5
TWO23 = 12582912.0  # 3*2^22: rounding shift
SHAPES = [(92, 92), (46, 46), (23, 23), (12, 12)]
K = 2               # x-lerp quantization levels (idx must fit int16)
RG = 12288          # padded-grid rows per (s, m), multiple of 1024
R1 = sum((h + 2) * (w + 2) for h, w in SHAPES)  # 11961 real padded rows
NT = RG // 1024     # macro-tiles in grid build
LB = []             # level base rows (padded space)
_acc = 0
for _h, _w in SHAPES:
    LB.append(_acc)
    _acc += (_h + 2) * (_w + 2)

DEBUG = False
# heads whose blend-multiply runs on the Pool (gpsimd) engine
HEADS_ON_POOL = (0, 1, 2, 3, 4, 5)


def build_program():
    nc = bacc.Bacc("TRN2", target_bir_lowering=False, debug=False, enable_asserts=False)

    t = {}
    def inp(name, shape, dtype=F32):
        t[name] = nc.dram_tensor(name, list(shape), dtype, kind="ExternalInput").ap()
        return t[name]

    # per-core data
    inp("qf", (LCPAD, DM)); inp("qp", (LCPAD, DM)); inp("ref", (LCPAD, LV, 2))
    inp("srcb", (L, DM))
    # weights (bf16 on host for matmul rhs)
    inp("Woff", (DM, M * LV * P * 2), BF16)
    inp("Wattn", (DM, M * LV * P), BF16)
    inp("Wv", (DM, DM), BF16)
    inp("Wout", (DM, DM), BF16)
    inp("W1", (DM, DF), BF16)
    inp("W2", (DF, DM), BF16)
    # biases as [1, N] rows (rank-1 matmul trick), bf16
    inp("boff", (1, M * LV * P * 2), BF16)
    inp("battn", (1, M * LV * P), BF16)
    inp("bv", (1, DM), BF16)
    inp("bout", (1, DM), BF16)
    inp("b1", (1, DF), BF16)
    inp("b2", (1, DM), BF16)
    # layernorm params replicated across partitions (f32)
    inp("ln1g", (128, DM)); inp("ln1b", (128, DM))
    inp("ln2g", (128, DM)); inp("ln2b", (128, DM))
    # constants
    inp("ident", (128, 128))              # f32 identity for PE transpose
    inp("ones1", (1, 128), BF16)          # rank-1 lhsT of ones
    inp("refdims", (128, LV * 2))         # (l,xy) -> W_l | H_l, replicated
    inp("dims", (128, M * LV * P * 2))    # (m,l,p,xy) -> W_l | H_l
    inp("dimm1y", (128, M * LV * P))      # (m,l,p) -> H_l - 1
    inp("h2t", (128, M * LV * P))         # (m,l,p) -> H_l + 2 (col-major x stride)
    inp("cidx", (128, M * LV * P))        # (m,l,p) -> m*RG + LB_l + (H_l+2) + 1

    out_ap = nc.dram_tensor("out", [LCPAD, DM], F32, kind="ExternalOutput").ap()
    if DEBUG:
        for nm, w in (("dbg_sampled", 256), ("dbg_aw", 128), ("dbg_idx", 128),
                      ("dbg_sw", 256), ("dbg_x", 256)):
            t[nm] = nc.dram_tensor(nm, [LCPAD, w], F32, kind="ExternalOutput").ap()

    with tile.TileContext(nc) as tc:
        _build(tc, out_ap, t)

    nc.compile()
    return nc


def _build(tc, out_ap, t):
    nc = tc.nc
    from contextlib import ExitStack
    ctx = ExitStack()
    with ctx:
        consts = ctx.enter_context(tc.tile_pool(name="consts", bufs=1))
        wpool = ctx.enter_context(tc.tile_pool(name="wpool", bufs=1))
        vblk = ctx.enter_context(tc.tile_pool(name="vblk", bufs=3))
        gblk = ctx.enter_context(tc.tile_pool(name="gblk", bufs=2))
        blk = ctx.enter_context(tc.tile_pool(name="blk", bufs=2))
        gpool = ctx.enter_context(tc.tile_pool(name="gpool", bufs=4))
        ps_t = ctx.enter_context(tc.tile_pool(name="ps_t", bufs=2, space="PSUM"))
        ps_mm = ctx.enter_context(tc.tile_pool(name="ps_mm", bufs=2, space="PSUM"))
        ps_f1 = ctx.enter_context(tc.tile_pool(name="ps_f1", bufs=2, space="PSUM"))
        dram = ctx.enter_context(tc.tile_pool(name="dram", bufs=1, space="DRAM"))

        # ---- resident constants / weights in SBUF ----
        def ld(name):
            ap = t[name]
            tile_ = consts.tile(list(ap.shape), ap.dtype, name=name + "_s")
            nc.sync.dma_start(out=tile_, in_=ap)
            return tile_

        ident = ld("ident")
        ones1 = ld("ones1")
        eps_t = consts.tile([128, 1], F32, name="eps_t")
        nc.vector.memset(eps_t, EPS)
        refdims = ld("refdims"); dims = ld("dims")
        dimm1y = ld("dimm1y"); h2t = ld("h2t"); cidx = ld("cidx")
        ln1g = ld("ln1g"); ln1b = ld("ln1b"); ln2g = ld("ln2g"); ln2b = ld("ln2b")
        boff = ld("boff"); battn = ld("battn"); bv = ld("bv")
        bout = ld("bout"); b1 = ld("b1"); b2 = ld("b2")

        def ldw(name, kchunks):
            ap = t[name]
            K_, N = ap.shape
            tiles = []
            for k in range(kchunks):
                w_ = wpool.tile([128, N], ap.dtype, name=f"{name}_{k}")
                nc.sync.dma_start(out=w_, in_=ap[k * 128:(k + 1) * 128, :])
                tiles.append(w_)
            return tiles

        Woff = ldw("Woff", 2); Wattn = ldw("Wattn", 2); Wv = ldw("Wv", 2)
        Wout = ldw("Wout", 2); W1 = ldw("W1", 2); W2 = ldw("W2", 8)

        # ---------------------------------------------------------------
        # Phase 1: value projection -> value1 (bf16, [L, M*DH]) in DRAM
        # ---------------------------------------------------------------
        value1 = dram.tile([L, DM], BF16, name="value1")

        for vb in range((L + 127) // 128):
            p0 = vb * 128
            pn = min(128, L - p0)
            s_t = vblk.tile([128, DM], F32, name="s_t")
            nc.sync.dma_start(out=s_t[:pn], in_=t["srcb"][p0:p0 + pn, :])
            sT = []
            for c in range(2):
                tp = ps_t.tile([128, 128], F32, name="v_tp", tag="tp")
                nc.tensor.transpose(out=tp[:, :pn], in_=s_t[:pn, c * 128:(c + 1) * 128],
                                    identity=ident[:pn, :pn])
                sb = vblk.tile([128, 128], BF16, name="v_sT")
                nc.scalar.activation(out=sb[:, :pn], in_=tp[:, :pn], func=AF.Copy)
                sT.append(sb)
            pv = ps_mm.tile([128, DM], F32, name="v_ps", tag="mm")
            for c in range(2):
                nc.tensor.matmul(out=pv[:pn], lhsT=sT[c][:, :pn], rhs=Wv[c],
                                 start=(c == 0), stop=False)
            nc.tensor.matmul(out=pv[:pn], lhsT=ones1[:, :pn], rhs=bv, start=False, stop=True)
            vt = vblk.tile([128, DM], BF16, name="v_out")
            nc.scalar.activation(out=vt[:pn], in_=pv[:pn], func=AF.Copy)
            nc.sync.dma_start(out=value1[p0:p0 + pn, :], in_=vt[:pn])

        # ---------------------------------------------------------------
        # Phase 2: padded grids (COLUMN-major per level: row = x'*(H+2)+y')
        # so that consecutive grid rows are y-neighbors (gathered as one
        # 128B run), then K x-lerped grids G.
        # ---------------------------------------------------------------
        v1p = dram.tile([RG, DM], BF16, name="v1p")
        v1px = dram.tile([RG, DM], BF16, name="v1px")  # v1p shifted by one x
        zt = vblk.tile([128, DM], BF16, name="zt")
        nc.vector.memset(zt, 0.0)
        for dst_t in (v1p, v1px):
            for zb in range(RG // 128):
                p0 = zb * 128
                nc.sync.dma_start(out=dst_t[p0:p0 + 128, :], in_=zt)

        # interior copies per level:
        # v1p[LB + (x+1)*(H+2) + (y+1)] = value1[lsi + y*W + x]
        lsi = 0
        for li, (H, W) in enumerate(SHAPES):
            src = value1[:].rearrange("r c -> (r c)")[
                lsi * DM:(lsi + H * W) * DM].rearrange("(y x c) -> y x c", y=H, x=W)
            _d0 = (LB[li] + (H + 2) + 1) * DM
            dst = v1p[:].rearrange("r c -> (r c)")[
                _d0:_d0 + W * (H + 2) * DM].rearrange(
                    "(x q) -> x q", x=W)[:, :H * DM].rearrange(
                    "x (y c) -> y x c", y=H)
            nc.sync.dma_start(out=dst, in_=src)
            lsi += H * W

        # v1px[r] = v1p[r + (H_l+2)] within each level block (x-shift)
        for li, (H, W) in enumerate(SHAPES):
            bs = (H + 2) * (W + 2)
            sh = H + 2
            src = v1p[:].rearrange("r c -> (r c)")[
                (LB[li] + sh) * DM:(LB[li] + bs) * DM]
            dst = v1px[:].rearrange("r c -> (r c)")[
                LB[li] * DM:(LB[li] + bs - sh) * DM]
            nc.sync.dma_start(out=dst, in_=src)

        # G2: K x-lerped, y-pair-duplicated grids, f32, one leading pad row.
        # Row 1 + s*RG + r holds, per head, [lerp_s(r), lerp_s(r+1)]
        # (column-major => r+1 is the y-neighbor). A sample point is one
        # 256B run: G2[1 + s*RG + row, m, :, :].
        RB = M * 2 * DH  # row elems (512)
        G2 = dram.tile([K * RG + 1, RB], F32, name="G2")
        g2f = G2[:].rearrange("r c -> (r c)")
        for it in range(NT):
            r0 = it * 1024
            t0 = gblk.tile([128, 8 * DM], BF16, name="g_t0")
            t1 = gblk.tile([128, 8 * DM], BF16, name="g_t1")
            nc.sync.dma_start(
                out=t0, in_=v1p[r0:r0 + 1024, :].rearrange("(p j) c -> p (j c)", p=128))
            nc.sync.dma_start(
                out=t1, in_=v1px[r0:r0 + 1024, :].rearrange("(p j) c -> p (j c)", p=128))
            d = gblk.tile([128, 8 * DM], BF16, name="g_d")
            nc.vector.tensor_tensor(out=d, in0=t1, in1=t0, op=OP.subtract)
            da = gblk.tile([128, 8 * DM], BF16, name="g_da")
            for s in range(K):
                gs = gblk.tile([128, 8 * DM], F32, name="g_gs")
                if s == 0:
                    nc.vector.tensor_copy(out=gs, in_=t0)
                else:
                    nc.vector.tensor_scalar(out=da, in0=d, scalar1=float(s) / K,
                                            scalar2=None, op0=OP.mult)
                    nc.vector.tensor_tensor(out=gs, in0=da, in1=t0, op=OP.add)
                # gs row (p*8+j) -> G2[1 + s*RG + r0 + p*8+j - dy][m][dy][:]
                # split into 16-partition chunks (1024 descriptors each)
                for dy in range(2):
                    for jc in range(8):
                        src = gs[jc * 16:(jc + 1) * 16, :].rearrange(
                            "p (j m c) -> p j m c", m=M, j=8)
                        off = (1 + s * RG + r0 + jc * 128 - dy) * RB + dy * DH
                        dst = g2f[off:off + 128 * RB].rearrange(
                            "(p j q) -> p j q", p=16, j=8).rearrange(
                            "p j (m y c) -> p j m y c", m=M, y=2)[:, :, :, 0, :]
                        nc.sync.dma_start(out=dst, in_=src)

        G2v = G2[:].rearrange("r (m q) -> r m q", m=M)  # [K*RG+1, M, 2*DH]

        # ---------------------------------------------------------------
        # Phase 3: main per-block loop
        # ---------------------------------------------------------------
        for ib in range(NBLK):
            q0 = ib * 128
            qf_t = blk.tile([128, DM], F32, name="qf_t")
            qp_t = blk.tile([128, DM], F32, name="qp_t")
            ref_t = blk.tile([128, LV, 2], F32, name="ref_t")
            nc.sync.dma_start(out=qf_t, in_=t["qf"][q0:q0 + 128, :])
            nc.sync.dma_start(out=qp_t, in_=t["qp"][q0:q0 + 128, :])
            nc.sync.dma_start(out=ref_t, in_=t["ref"][q0:q0 + 128, :, :])

            # transposes of qf and qp (query = qf + qp folded into matmuls)
            qT = []
            for src_t in (qf_t, qp_t):
                for c in range(2):
                    tp = ps_t.tile([128, 128], F32, name="q_tp", tag="tp")
                    nc.tensor.transpose(out=tp, in_=src_t[:, c * 128:(c + 1) * 128],
                                        identity=ident)
                    sb = blk.tile([128, 128], BF16, name="qT")
                    nc.scalar.activation(out=sb, in_=tp, func=AF.Copy)
                    qT.append(sb)
            # qT = [qf0, qf1, qp0, qp1]

            # offsets projection [128q, 256] (m,l,p,xy)
            ps_off = ps_mm.tile([128, 256], F32, name="ps_off", tag="mm")
            for i, w_ in ((0, Woff[0]), (1, Woff[1]), (2, Woff[0]), (3, Woff[1])):
                nc.tensor.matmul(out=ps_off, lhsT=qT[i], rhs=w_, start=(i == 0), stop=False)
            nc.tensor.matmul(out=ps_off, lhsT=ones1, rhs=boff, start=False, stop=True)

            # attention weights projection + softmax over (l,p) per head
            ps_at = ps_mm.tile([128, 128], F32, name="ps_at", tag="mm")
            for i, w_ in ((0, Wattn[0]), (1, Wattn[1]), (2, Wattn[0]), (3, Wattn[1])):
                nc.tensor.matmul(out=ps_at, lhsT=qT[i], rhs=w_, start=(i == 0), stop=False)
            nc.tensor.matmul(out=ps_at, lhsT=ones1, rhs=battn, start=False, stop=True)
            expt = blk.tile([128, 128], F32, name="expt")
            nc.scalar.activation(out=expt, in_=ps_at, func=AF.Exp)
            den = blk.tile([128, M], F32, name="den")
            nc.vector.tensor_reduce(out=den, in_=expt[:].rearrange("p (m k) -> p m k", m=M),
                                    axis=AX.X, op=OP.add)
            nc.vector.reciprocal(out=den, in_=den)
            aw = blk.tile([128, 128], F32, name="aw")
            nc.vector.tensor_tensor(out=aw[:].rearrange("p (m k) -> p m k", m=M),
                                    in0=expt[:].rearrange("p (m k) -> p m k", m=M),
                                    in1=den[:, :, None].broadcast_to([128, M, LV * P]),
                                    op=OP.mult)

            # ---- sampling coordinates ----
            # refe[l, xy] = ref * (W|H) - 0.5  (small [128, 8] ops)
            refe = blk.tile([128, LV * 2], F32, name="refe")
            nc.vector.tensor_tensor(out=refe, in0=ref_t[:].rearrange("p l x -> p (l x)"),
                                    in1=refdims, op=OP.mult)
            nc.vector.tensor_scalar(out=refe, in0=refe, scalar1=0.5, scalar2=None,
                                    op0=OP.subtract)
            # expand refe (l,xy) -> (l,p,xy), then x = off + refe32 (bcast m)
            refe32 = blk.tile([128, LV * P * 2], F32, name="refe32")
            nc.vector.tensor_copy(
                out=refe32[:].rearrange("p (l q y) -> p l q y", l=LV, q=P),
                in_=refe[:].rearrange("p (l y) -> p l y", l=LV)[:, :, None, :]
                    .broadcast_to([128, LV, P, 2]))
            x = blk.tile([128, 256], F32, name="x")
            nc.vector.tensor_tensor(
                out=x[:].rearrange("p (m k) -> p m k", m=M),
                in0=ps_off[:].rearrange("p (m k) -> p m k", m=M),
                in1=refe32[:, None, :].broadcast_to([128, M, LV * P * 2]),
                op=OP.add)
            # clip to [-1, dim]
            nc.vector.tensor_scalar(out=x, in0=x, scalar1=-1.0, scalar2=None, op0=OP.max)
            nc.vector.tensor_tensor(out=x, in0=x, in1=dims, op=OP.min)

            xv = x[:].rearrange("p (k y) -> p k y", y=2)
            xc = xv[:, :, 0]   # [128, 128] strided views
            yc = xv[:, :, 1]

            # xQ = round(K * xc)
            xQ = blk.tile([128, 128], F32, name="xQ")
            nc.vector.tensor_scalar(out=xQ, in0=xc, scalar1=float(K), scalar2=TWO23,
                                    op0=OP.mult, op1=OP.add)
            nc.vector.tensor_scalar(out=xQ, in0=xQ, scalar1=TWO23, scalar2=None,
                                    op0=OP.subtract)
            # x0 = floor(xQ / K) = round(xQ/K - (K-1)/(2K)), tie-free
            x0 = blk.tile([128, 128], F32, name="x0")
            nc.vector.tensor_scalar(out=x0, in0=xQ, scalar1=1.0 / K,
                                    scalar2=(K - 1.0) / (2 * K),
                                    op0=OP.mult, op1=OP.subtract)
            nc.vector.tensor_scalar(out=x0, in0=x0, scalar1=TWO23, scalar2=TWO23,
                                    op0=OP.add, op1=OP.subtract)
            # y0 = clip(round(yc - 0.5), -1, H-1); wy = yc - y0
            y0 = blk.tile([128, 128], F32, name="y0")
            nc.vector.tensor_scalar(out=y0, in0=yc, scalar1=0.5, scalar2=TWO23,
                                    op0=OP.subtract, op1=OP.add)
            nc.vector.tensor_scalar(out=y0, in0=y0, scalar1=TWO23, scalar2=-1.0,
                                    op0=OP.subtract, op1=OP.max)
            nc.vector.tensor_tensor(out=y0, in0=y0, in1=dimm1y, op=OP.min)
            wy = blk.tile([128, 128], F32, name="wy")
            nc.vector.tensor_tensor(out=wy, in0=yc, in1=y0, op=OP.subtract)

            # sw[q, (m,l,p), y] : y0-weight = aw*(1-wy), y1-weight = aw*wy (bf16)
            sw = blk.tile([128, 256], BF16, name="sw")
            swv = sw[:].rearrange("p (k y) -> p k y", y=2)
            nc.vector.tensor_tensor(out=swv[:, :, 1], in0=aw, in1=wy, op=OP.mult)
            nc.vector.tensor_tensor(out=swv[:, :, 0], in0=aw, in1=swv[:, :, 1],
                                    op=OP.subtract)

            # idx = (s*M + m)*RG + LB_l + (x0+1)*(H+2) + (y0+1), s = xQ - K*x0
            s_t2 = blk.tile([128, 128], F32, name="s_t2")
            nc.vector.scalar_tensor_tensor(out=s_t2, in0=x0, scalar=-float(K), in1=xQ,
                                           op0=OP.mult, op1=OP.add)
            idxf = blk.tile([128, 128], F32, name="idxf")
            nc.vector.tensor_tensor(out=idxf, in0=x0, in1=h2t, op=OP.mult)
            nc.vector.tensor_tensor(out=idxf, in0=idxf, in1=y0, op=OP.add)
            nc.vector.tensor_tensor(out=idxf, in0=idxf, in1=cidx, op=OP.add)
            nc.vector.scalar_tensor_tensor(out=idxf, in0=s_t2, scalar=float(RG),
                                           in1=idxf, op0=OP.mult, op1=OP.add)
            idx16 = blk.tile([128, 128], I16, name="idx16")
            nc.vector.tensor_copy(out=idx16, in_=idxf)
            # wrap indices into dma_gather layout: fold2r[qlo, (m, pt, qhi)]
            fold1 = blk.tile([16, 8, 128], I16, name="fold1")
            for qhi in range(8):
                nc.sync.dma_start(out=fold1[:, qhi, :],
                                  in_=idx16[qhi * 16:(qhi + 1) * 16, :])
            fold2r = blk.tile([128, M * 128], I16, name="fold2r")
            nc.vector.tensor_copy(
                out=fold2r[0:16, :].rearrange("p (m k q) -> p m k q", m=M, k=16),
                in_=fold1[:].rearrange("p q (m k) -> p m k q", m=M))
            nc.sync.dma_start(out=fold2r[16:32, :], in_=fold2r[0:16, :])
            nc.sync.dma_start(out=fold2r[32:64, :], in_=fold2r[0:32, :])
            nc.sync.dma_start(out=fold2r[64:128, :], in_=fold2r[0:64, :])

            if DEBUG:
                nc.sync.dma_start(out=t["dbg_aw"][q0:q0 + 128, :], in_=aw)
                nc.sync.dma_start(out=t["dbg_idx"][q0:q0 + 128, :], in_=idxf)
                nc.sync.dma_start(out=t["dbg_x"][q0:q0 + 128, :], in_=x)
                dbg_sw = blk.tile([128, 256], F32, name="dbg_sw")
                nc.vector.tensor_copy(out=dbg_sw, in_=sw)
                nc.sync.dma_start(out=t["dbg_sw"][q0:q0 + 128, :], in_=dbg_sw)

            # ---- gather + blend per head ----
            sampled = blk.tile([128, DM], F32, name="sampled")
            for m in range(M):
                g2m = gpool.tile([128, 16, 2 * DH], F32, name="g2m")
                for hk in range(2):
                    nc.gpsimd.dma_gather(
                        out_ap=g2m[:, hk * 8:(hk + 1) * 8, :],
                        in_ap=G2v[:, m, :],
                        idxs_ap=fold2r[:, m * 128 + hk * 64:m * 128 + (hk + 1) * 64],
                        num_idxs=1024, num_idxs_reg=1024,
                        elem_size=2 * DH, elem_step=RB,
                        transpose=False, queue_num=0)
                wtm = gpool.tile([128, LV * P * 2 * DH], BF16, name="wtm")
                eng = nc.gpsimd if m in HEADS_ON_POOL else nc.vector
                eng.tensor_tensor(
                    out=wtm[:].rearrange("p (k y c) -> p k y c", y=2, c=DH),
                    in0=g2m[:].rearrange("p k (y c) -> p k y c", y=2),
                    in1=sw[:].rearrange("p (k y) -> p k y", y=2)[
                        :, m * 16:(m + 1) * 16, :, None]
                        .broadcast_to([128, 16, 2, DH]),
                    op=OP.mult)
                nc.vector.tensor_reduce(
                    out=sampled[:, m * DH:(m + 1) * DH],
                    in_=wtm[:].rearrange("p (u c) -> p c u", c=DH),
                    axis=AX.X, op=OP.add)

            if DEBUG:
                nc.sync.dma_start(out=t["dbg_sampled"][q0:q0 + 128, :], in_=sampled)

            # ---- output projection ----
            sT = []
            for c in range(2):
                tp = ps_t.tile([128, 128], F32, name="s_tp", tag="tp")
                nc.tensor.transpose(out=tp, in_=sampled[:, c * 128:(c + 1) * 128],
                                    identity=ident)
                sb = blk.tile([128, 128], BF16, name="sT")
                nc.scalar.activation(out=sb, in_=tp, func=AF.Copy)
                sT.append(sb)
            ps_h = ps_mm.tile([128, DM], F32, name="ps_h", tag="mm")
            for c in range(2):
                nc.tensor.matmul(out=ps_h, lhsT=sT[c], rhs=Wout[c], start=(c == 0), stop=False)
            nc.tensor.matmul(out=ps_h, lhsT=ones1, rhs=bout, start=False, stop=True)

            # ---- LN1 ----
            r1 = blk.tile([128, DM], F32, name="r1")
            nc.vector.tensor_tensor(out=r1, in0=qf_t, in1=ps_h, op=OP.add)
            h = _layernorm(nc, blk, r1, ln1g, ln1b, eps_t)

            # ---- FFN ----
            hT = []
            for c in range(2):
                tp = ps_t.tile([128, 128], F32, name="h_tp", tag="tp")
                nc.tensor.transpose(out=tp, in_=h[:, c * 128:(c + 1) * 128], identity=ident)
                sb = blk.tile([128, 128], BF16, name="hT")
                nc.scalar.activation(out=sb, in_=tp, func=AF.Copy)
                hT.append(sb)
            relu1 = []
            for fc in range(8):
                pf = ps_f1.tile([128, 128], F32, name="pf")
                for c in range(2):
                    nc.tensor.matmul(out=pf, lhsT=W1[c][:, fc * 128:(fc + 1) * 128],
                                     rhs=hT[c], start=(c == 0), stop=False)
                nc.tensor.matmul(out=pf, lhsT=b1[:, fc * 128:(fc + 1) * 128],
                                 rhs=ones1, start=False, stop=True)
                rt = blk.tile([128, 128], BF16, name=f"relu1_{fc}")
                nc.scalar.activation(out=rt, in_=pf, func=AF.Relu)
                relu1.append(rt)
            ps_o = ps_mm.tile([128, DM], F32, name="ps_o", tag="mm")
            for fc in range(8):
                nc.tensor.matmul(out=ps_o, lhsT=relu1[fc], rhs=W2[fc], start=(fc == 0),
                                 stop=False)
            nc.tensor.matmul(out=ps_o, lhsT=ones1, rhs=b2, start=False, stop=True)

            # ---- LN2 + store ----
            r2 = blk.tile([128, DM], F32, name="r2")
            nc.vector.tensor_tensor(out=r2, in0=h, in1=ps_o, op=OP.add)
            o = _layernorm(nc, blk, r2, ln2g, ln2b, eps_t)
            nc.sync.dma_start(out=out_ap[q0:q0 + 128, :], in_=o)


def _layernorm(nc, pool, r, g, b, eps_t):
    stats = pool.tile([128, 6], F32, name="ln_stats")
    nc.vector.bn_stats(out=stats, in_=r)
    mv = pool.tile([128, 2], F32, name="ln_mv")
    nc.vector.bn_aggr(out=mv, in_=stats)
    rstd = pool.tile([128, 1], F32, name="ln_rstd")
    nc.scalar.activation(out=rstd, in_=mv[:, 1:2], func=AF.Sqrt, bias=eps_t)
    nc.vector.reciprocal(out=rstd, in_=rstd)
    xs = pool.tile([128, DM], F32, name="ln_xs")
    nc.vector.tensor_scalar(out=xs, in0=r, scalar1=mv[:, 0:1], scalar2=rstd,
                            op0=OP.subtract, op1=OP.mult)
    h = pool.tile([128, DM], F32, name="ln_h")
    nc.vector.tensor_tensor(out=h, in0=xs, in1=g, op=OP.mult)
    nc.vector.tensor_tensor(out=h, in0=h, in1=b, op=OP.add)
    return h


# ---------------------------------------------------------------------------
# host side
# ---------------------------------------------------------------------------

_prog_cache = {}


def _get_program():
    if "nc" not in _prog_cache:
        _prog_cache["nc"] = build_program()
    return _prog_cache["nc"]


def _host_constants():
    f = np.float32
    H = np.array([h for h, w in SHAPES], np.int64)
    W = np.array([w for h, w in SHAPES], np.int64)
    # refdims [(l, xy)]: xy=0 -> W, xy=1 -> H
    refd = np.zeros((LV, 2), f)
    refd[:, 0] = W; refd[:, 1] = H
    refdims = np.broadcast_to(refd.reshape(1, -1), (128, LV * 2)).copy()
    # dims [(m,l,p,xy)] -> W | H (clip max)
    dm = np.zeros((M, LV, P, 2), f)
    dm[:, :, :, 0] = W[None, :, None]
    dm[:, :, :, 1] = H[None, :, None]
    dims = np.broadcast_to(dm.reshape(1, -1), (128, M * LV * P * 2)).copy()
    # dimm1y [(m,l,p)] -> H - 1
    d1 = np.zeros((M, LV, P), f)
    d1[:, :, :] = (H - 1)[None, :, None]
    dimm1y = np.broadcast_to(d1.reshape(1, -1), (128, M * LV * P)).copy()
    # h2t [(m,l,p)] -> H + 2 (column-major x-stride)
    h2 = np.zeros((M, LV, P), f)
    h2[:, :, :] = (H + 2)[None, :, None]
    h2t = np.broadcast_to(h2.reshape(1, -1), (128, M * LV * P)).copy()
    # cidx [(m,l,p)] -> LB_l + (H_l+2) + 1 + 1 (one leading G2 pad row)
    ci = np.zeros((M, LV, P), f)
    for m in range(M):
        for li in range(LV):
            ci[m, li, :] = LB[li] + (H[li] + 2) + 2
    cidx = np.broadcast_to(ci.reshape(1, -1), (128, M * LV * P)).copy()
    return refdims, dims, dimm1y, h2t, cidx


def _build_in_maps(inputs):
    src = np.asarray(inputs["src"], np.float32)
    q_feat = np.asarray(inputs["q_feat"], np.float32)
    q_pos = np.asarray(inputs["q_pos"], np.float32)
    ref = np.asarray(inputs["reference_points"], np.float32)
    ss = np.asarray(inputs["spatial_shapes"])
    lsi_in = np.asarray(inputs["level_start_index"])
    assert src.shape == (B, L, DM), src.shape
    assert [tuple(r) for r in ss.tolist()] == SHAPES, ss
    assert lsi_in.tolist() == [0, 8464, 10580, 11109], lsi_in

    refdims, dims, dimm1y, h2t, cidx = _host_constants()

    def as_bf16(a):
        import ml_dtypes
        return np.asarray(a, np.float32).astype(ml_dtypes.bfloat16)

    common = {
        "Woff": as_bf16(inputs["W_off"]),
        "Wattn": as_bf16(inputs["W_attn"]),
        "Wv": as_bf16(inputs["W_v"]),
        "Wout": as_bf16(inputs["W_out"]),
        "W1": as_bf16(inputs["W1"]),
        "W2": as_bf16(inputs["W2"]),
        "boff": as_bf16(inputs["b_off"]).reshape(1, -1),
        "battn": as_bf16(inputs["b_attn"]).reshape(1, -1),
        "bv": as_bf16(inputs["b_v"]).reshape(1, -1),
        "bout": as_bf16(inputs["b_out"]).reshape(1, -1),
        "b1": as_bf16(inputs["b1"]).reshape(1, -1),
        "b2": as_bf16(inputs["b2"]).reshape(1, -1),
        "ln1g": np.broadcast_to(np.asarray(inputs["ln1_g"], np.float32), (128, DM)).copy(),
        "ln1b": np.broadcast_to(np.asarray(inputs["ln1_b"], np.float32), (128, DM)).copy(),
        "ln2g": np.broadcast_to(np.asarray(inputs["ln2_g"], np.float32), (128, DM)).copy(),
        "ln2b": np.broadcast_to(np.asarray(inputs["ln2_b"], np.float32), (128, DM)).copy(),
        "ident": np.eye(128, dtype=np.float32),
        "ones1": as_bf16(np.ones((1, 128), np.float32)),
        "refdims": refdims, "dims": dims, "dimm1y": dimm1y, "h2t": h2t, "cidx": cidx,
    }

    halves = [(0, LC), (LC, L - LC)]
    in_maps = []
    for core in range(8):
        b = core // 2
        h0, hn = halves[core % 2]
        qf = np.zeros((LCPAD, DM), np.float32)
        qp = np.zeros((LCPAD, DM), np.float32)
        rf = np.zeros((LCPAD, LV, 2), np.float32)
        qf[:hn] = q_feat[b, h0:h0 + hn]
        qp[:hn] = q_pos[b, h0:h0 + hn]
        rf[:hn] = ref[b, h0:h0 + hn]
        m = dict(common)
        m.update({"qf": qf, "qp": qp, "ref": rf, "srcb": src[b]})
        in_maps.append(m)
    return in_maps


def kernel(**inputs):
    from concourse.bass_utils import run_bass_kernel_spmd

    in_maps = _build_in_maps(inputs)
    nc = _get_program()
    res = run_bass_kernel_spmd(nc, in_maps, core_ids=list(range(8)))

    halves = [(0, LC), (LC, L - LC)]
    out = np.zeros((B, L, DM), np.float32)
    for core in range(8):
        b = core // 2
        h0, hn = halves[core % 2]
        out[b, h0:h0 + hn] = res.results[core]["out"][:hn]
    return out


# revision 25
# speedup vs baseline: 1.1166x; 1.1166x over previous
"""Deformable Transformer Encoder Layer — Trainium2 Bass kernel (v2).

Sharding: 8 cores = (batch b in 0..3) x (query-half h in 0..1).
Each core computes the full layer for its (b, query-half) slice.

Sampling strategy (per core):
  - value projection over the FULL batch image (PE), stored bf16.
  - zero-PADDED per-level grid v1p ((H+2)x(W+2) rows per level) makes
    grid_sample zero-padding semantics exact with plain clip+floor math
    (no OOB masks).
  - K=4 x-pre-lerped grids G_s = (1-s/4) v1p + (s/4) shift_x(v1p),
    laid out [s][m][row][32ch] bf16, so one sample point needs a single
    contiguous 128B run (y-row pair) fetched via gpsimd indirect DMA.
    x-fraction is quantized to 1/4 pixel (error ~1e-3 of value scale).
  - blend: per head, multiply gathered pairs by combined
    attention*(1-wy, wy) weights (Pool engine) and a segmented reduce
    (DVE), followed by out-proj / LN / FFN / LN as in the baseline.
No cross-core communication; host reassembles the output.
"""

import os
import sys
import numpy as np

for _p in ("/opt/trn_rl_repo", "/root/.axon_site/_ro/trn_rl_repo"):
    if os.path.isdir(_p) and _p not in sys.path:
        sys.path.insert(0, _p)

import concourse.bass as bass
import concourse.mybir as mybir
import concourse.tile as tile
from concourse import bacc
from concourse.bass import AP

F32 = mybir.dt.float32
BF16 = mybir.dt.bfloat16
I32 = mybir.dt.int32
I16 = mybir.dt.int16
AF = mybir.ActivationFunctionType
OP = mybir.AluOpType
AX = mybir.AxisListType

# Problem constants (checked against inputs at runtime on host).
M, LV, P, DM, DH, DF = 8, 4, 4, 256, 32, 1024
L = 11253
B = 4
LC = 5627           # queries per core (split [5627, 5626])
LCPAD = 5632        # 44 * 128
NBLK = LCPAD // 128
EPS = 1e-5
TWO23 = 12582912.0  # 3*2^22: rounding shift
SHAPES = [(92, 92), (46, 46), (23, 23), (12, 12)]
K = 2               # x-lerp quantization levels (idx must fit int16)
RG = 12288          # padded-grid rows per (s, m), multiple of 1024
R1 = sum((h + 2) * (w + 2) for h, w in SHAPES)  # 11961 real padded rows
NT = RG // 1024     # macro-tiles in grid build
LB = []             # level base rows (padded space)
_acc = 0
for _h, _w in SHAPES:
    LB.append(_acc)
    _acc += (_h + 2) * (_w + 2)

DEBUG = False
# heads whose blend-multiply runs on the Pool (gpsimd) engine
HEADS_ON_POOL = (0, 1, 2, 3)


def build_program():
    nc = bacc.Bacc("TRN2", target_bir_lowering=False, debug=False, enable_asserts=False)

    t = {}
    def inp(name, shape, dtype=F32):
        t[name] = nc.dram_tensor(name, list(shape), dtype, kind="ExternalInput").ap()
        return t[name]

    # per-core data
    inp("qf", (LCPAD, DM)); inp("qp", (LCPAD, DM)); inp("ref", (LCPAD, LV, 2))
    inp("srcb", (L, DM))
    # weights (bf16 on host for matmul rhs)
    inp("Woff", (DM, M * LV * P * 2), BF16)
    inp("Wattn", (DM, M * LV * P), BF16)
    inp("Wv", (DM, DM), BF16)
    inp("Wout", (DM, DM), BF16)
    inp("W1", (DM, DF), BF16)
    inp("W2", (DF, DM), BF16)
    # biases as [1, N] rows (rank-1 matmul trick), bf16
    inp("boff", (1, M * LV * P * 2), BF16)
    inp("battn", (1, M * LV * P), BF16)
    inp("bv", (1, DM), BF16)
    inp("bout", (1, DM), BF16)
    inp("b1", (1, DF), BF16)
    inp("b2", (1, DM), BF16)
    # layernorm params replicated across partitions (f32)
    inp("ln1g", (128, DM)); inp("ln1b", (128, DM))
    inp("ln2g", (128, DM)); inp("ln2b", (128, DM))
    # constants
    inp("ident", (128, 128))              # f32 identity for PE transpose
    inp("ones1", (1, 128), BF16)          # rank-1 lhsT of ones
    inp("refdims", (128, LV * 2))         # (l,xy) -> W_l | H_l, replicated
    inp("dims", (128, M * LV * P * 2))    # (m,l,p,xy) -> W_l | H_l
    inp("dimm1y", (128, M * LV * P))      # (m,l,p) -> H_l - 1
    inp("h2t", (128, M * LV * P))         # (m,l,p) -> H_l + 2 (col-major x stride)
    inp("cidx", (128, M * LV * P))        # (m,l,p) -> m*RG + LB_l + (H_l+2) + 1

    out_ap = nc.dram_tensor("out", [LCPAD, DM], F32, kind="ExternalOutput").ap()
    if DEBUG:
        for nm, w in (("dbg_sampled", 256), ("dbg_aw", 128), ("dbg_idx", 128),
                      ("dbg_sw", 256), ("dbg_x", 256)):
            t[nm] = nc.dram_tensor(nm, [LCPAD, w], F32, kind="ExternalOutput").ap()

    with tile.TileContext(nc) as tc:
        _build(tc, out_ap, t)

    nc.compile()
    return nc


def _build(tc, out_ap, t):
    nc = tc.nc
    from contextlib import ExitStack
    ctx = ExitStack()
    with ctx:
        consts = ctx.enter_context(tc.tile_pool(name="consts", bufs=1))
        wpool = ctx.enter_context(tc.tile_pool(name="wpool", bufs=1))
        vblk = ctx.enter_context(tc.tile_pool(name="vblk", bufs=3))
        gblk = ctx.enter_context(tc.tile_pool(name="gblk", bufs=2))
        blk = ctx.enter_context(tc.tile_pool(name="blk", bufs=3))
        gpool = ctx.enter_context(tc.tile_pool(name="gpool", bufs=6))
        ps_t = ctx.enter_context(tc.tile_pool(name="ps_t", bufs=2, space="PSUM"))
        ps_mm = ctx.enter_context(tc.tile_pool(name="ps_mm", bufs=3, space="PSUM"))
        ps_f1 = ctx.enter_context(tc.tile_pool(name="ps_f1", bufs=2, space="PSUM"))
        dram = ctx.enter_context(tc.tile_pool(name="dram", bufs=1, space="DRAM"))

        # ---- resident constants / weights in SBUF ----
        def ld(name):
            ap = t[name]
            tile_ = consts.tile(list(ap.shape), ap.dtype, name=name + "_s")
            nc.sync.dma_start(out=tile_, in_=ap)
            return tile_

        ident = ld("ident")
        ones1 = ld("ones1")
        eps_t = consts.tile([128, 1], F32, name="eps_t")
        nc.vector.memset(eps_t, EPS)
        refdims = ld("refdims"); dims = ld("dims")
        dimm1y = ld("dimm1y"); h2t = ld("h2t"); cidx = ld("cidx")
        ln1g = ld("ln1g"); ln1b = ld("ln1b"); ln2g = ld("ln2g"); ln2b = ld("ln2b")
        boff = ld("boff"); battn = ld("battn"); bv = ld("bv")
        bout = ld("bout"); b1 = ld("b1"); b2 = ld("b2")

        def ldw(name, kchunks):
            ap = t[name]
            K_, N = ap.shape
            tiles = []
            for k in range(kchunks):
                w_ = wpool.tile([128, N], ap.dtype, name=f"{name}_{k}")
                nc.sync.dma_start(out=w_, in_=ap[k * 128:(k + 1) * 128, :])
                tiles.append(w_)
            return tiles

        Woff = ldw("Woff", 2); Wattn = ldw("Wattn", 2); Wv = ldw("Wv", 2)
        Wout = ldw("Wout", 2); W1 = ldw("W1", 2); W2 = ldw("W2", 8)

        # ---------------------------------------------------------------
        # Phase 1: value projection -> value1 (bf16, [L, M*DH]) in DRAM
        # ---------------------------------------------------------------
        value1 = dram.tile([L, DM], BF16, name="value1")

        for vb in range((L + 127) // 128):
            p0 = vb * 128
            pn = min(128, L - p0)
            s_t = vblk.tile([128, DM], F32, name="s_t")
            nc.sync.dma_start(out=s_t[:pn], in_=t["srcb"][p0:p0 + pn, :])
            sT = []
            for c in range(2):
                tp = ps_t.tile([128, 128], F32, name="v_tp", tag="tp")
                nc.tensor.transpose(out=tp[:, :pn], in_=s_t[:pn, c * 128:(c + 1) * 128],
                                    identity=ident[:pn, :pn])
                sb = vblk.tile([128, 128], BF16, name="v_sT")
                nc.scalar.activation(out=sb[:, :pn], in_=tp[:, :pn], func=AF.Copy)
                sT.append(sb)
            pv = ps_mm.tile([128, DM], F32, name="v_ps", tag="mm")
            for c in range(2):
                nc.tensor.matmul(out=pv[:pn], lhsT=sT[c][:, :pn], rhs=Wv[c],
                                 start=(c == 0), stop=False)
            nc.tensor.matmul(out=pv[:pn], lhsT=ones1[:, :pn], rhs=bv, start=False, stop=True)
            vt = vblk.tile([128, DM], BF16, name="v_out")
            nc.scalar.activation(out=vt[:pn], in_=pv[:pn], func=AF.Copy)
            nc.sync.dma_start(out=value1[p0:p0 + pn, :], in_=vt[:pn])

        # ---------------------------------------------------------------
        # Phase 2: padded grids (COLUMN-major per level: row = x'*(H+2)+y')
        # so that consecutive grid rows are y-neighbors (gathered as one
        # 128B run), then K x-lerped grids G.
        # ---------------------------------------------------------------
        v1p = dram.tile([RG, DM], BF16, name="v1p")
        v1px = dram.tile([RG, DM], BF16, name="v1px")  # v1p shifted by one x
        zt = vblk.tile([128, DM], BF16, name="zt")
        nc.vector.memset(zt, 0.0)
        for zb in range(RG // 128):
            p0 = zb * 128
            nc.sync.dma_start(out=v1p[p0:p0 + 128, :], in_=zt)
        # v1px tails per level are unwritten; those rows only feed G2 rows
        # that are never gathered, but zero the last level tail for safety
        nc.sync.dma_start(out=v1px[RG - 128:RG, :], in_=zt)

        # interior copies per level:
        # v1p[LB + (x+1)*(H+2) + (y+1)] = value1[lsi + y*W + x]
        lsi = 0
        for li, (H, W) in enumerate(SHAPES):
            src = value1[:].rearrange("r c -> (r c)")[
                lsi * DM:(lsi + H * W) * DM].rearrange("(y x c) -> y x c", y=H, x=W)
            _d0 = (LB[li] + (H + 2) + 1) * DM
            dst = v1p[:].rearrange("r c -> (r c)")[
                _d0:_d0 + W * (H + 2) * DM].rearrange(
                    "(x q) -> x q", x=W)[:, :H * DM].rearrange(
                    "x (y c) -> y x c", y=H)
            nc.sync.dma_start(out=dst, in_=src)
            lsi += H * W

        # v1px[r] = v1p[r + (H_l+2)] within each level block (x-shift)
        for li, (H, W) in enumerate(SHAPES):
            bs = (H + 2) * (W + 2)
            sh = H + 2
            src = v1p[:].rearrange("r c -> (r c)")[
                (LB[li] + sh) * DM:(LB[li] + bs) * DM]
            dst = v1px[:].rearrange("r c -> (r c)")[
                LB[li] * DM:(LB[li] + bs - sh) * DM]
            nc.sync.dma_start(out=dst, in_=src)

        # G2: K x-lerped, y-pair-duplicated grids, f32, one leading pad row.
        # Row 1 + s*RG + r holds, per head, [lerp_s(r), lerp_s(r+1)]
        # (column-major => r+1 is the y-neighbor). A sample point is one
        # 256B run: G2[1 + s*RG + row, m, :, :].
        RB = M * 2 * DH  # row elems (512)
        G2 = dram.tile([K * RG + 1, RB], F32, name="G2")
        g2f = G2[:].rearrange("r c -> (r c)")
        for it in range(NT):
            r0 = it * 1024
            t0 = gblk.tile([128, 8 * DM], BF16, name="g_t0")
            t1 = gblk.tile([128, 8 * DM], BF16, name="g_t1")
            nc.sync.dma_start(
                out=t0, in_=v1p[r0:r0 + 1024, :].rearrange("(p j) c -> p (j c)", p=128))
            nc.sync.dma_start(
                out=t1, in_=v1px[r0:r0 + 1024, :].rearrange("(p j) c -> p (j c)", p=128))
            d = gblk.tile([128, 8 * DM], BF16, name="g_d")
            nc.vector.tensor_tensor(out=d, in0=t1, in1=t0, op=OP.subtract)
            da = gblk.tile([128, 8 * DM], BF16, name="g_da")
            for s in range(K):
                gs = gblk.tile([128, 8 * DM], F32, name="g_gs")
                if s == 0:
                    nc.vector.tensor_copy(out=gs, in_=t0)
                else:
                    nc.vector.tensor_scalar(out=da, in0=d, scalar1=float(s) / K,
                                            scalar2=None, op0=OP.mult)
                    nc.vector.tensor_tensor(out=gs, in0=da, in1=t0, op=OP.add)
                # gs row (p*8+j) -> G2[1 + s*RG + r0 + p*8+j - dy][m][dy][:]
                # split into 16-partition chunks (1024 descriptors each)
                for dy in range(2):
                    for jc in range(4):
                        src = gs[jc * 32:(jc + 1) * 32, :].rearrange(
                            "p (j m c) -> p j m c", m=M, j=8)
                        off = (1 + s * RG + r0 + jc * 256 - dy) * RB + dy * DH
                        dst = g2f[off:off + 256 * RB].rearrange(
                            "(p j q) -> p j q", p=32, j=8).rearrange(
                            "p j (m y c) -> p j m y c", m=M, y=2)[:, :, :, 0, :]
                        nc.sync.dma_start(out=dst, in_=src)

        G2v = G2[:].rearrange("r (m q) -> r m q", m=M)  # [K*RG+1, M, 2*DH]

        # ---------------------------------------------------------------
        # Phase 3: main per-block loop
        # ---------------------------------------------------------------
        for ib in range(NBLK):
            q0 = ib * 128
            qf_t = blk.tile([128, DM], F32, name="qf_t")
            qp_t = blk.tile([128, DM], F32, name="qp_t")
            ref_t = blk.tile([128, LV, 2], F32, name="ref_t")
            nc.sync.dma_start(out=qf_t, in_=t["qf"][q0:q0 + 128, :])
            nc.sync.dma_start(out=qp_t, in_=t["qp"][q0:q0 + 128, :])
            nc.sync.dma_start(out=ref_t, in_=t["ref"][q0:q0 + 128, :, :])

            # transposes of qf and qp (query = qf + qp folded into matmuls)
            qT = []
            for src_t in (qf_t, qp_t):
                for c in range(2):
                    tp = ps_t.tile([128, 128], F32, name="q_tp", tag="tp")
                    nc.tensor.transpose(out=tp, in_=src_t[:, c * 128:(c + 1) * 128],
                                        identity=ident)
                    sb = blk.tile([128, 128], BF16, name="qT")
                    nc.scalar.activation(out=sb, in_=tp, func=AF.Copy)
                    qT.append(sb)
            # qT = [qf0, qf1, qp0, qp1]

            # offsets projection [128q, 256] (m,l,p,xy)
            ps_off = ps_mm.tile([128, 256], F32, name="ps_off", tag="mm")
            for i, w_ in ((0, Woff[0]), (1, Woff[1]), (2, Woff[0]), (3, Woff[1])):
                nc.tensor.matmul(out=ps_off, lhsT=qT[i], rhs=w_, start=(i == 0), stop=False)
            nc.tensor.matmul(out=ps_off, lhsT=ones1, rhs=boff, start=False, stop=True)

            # attention weights projection + softmax over (l,p) per head
            ps_at = ps_mm.tile([128, 128], F32, name="ps_at", tag="mm")
            for i, w_ in ((0, Wattn[0]), (1, Wattn[1]), (2, Wattn[0]), (3, Wattn[1])):
                nc.tensor.matmul(out=ps_at, lhsT=qT[i], rhs=w_, start=(i == 0), stop=False)
            nc.tensor.matmul(out=ps_at, lhsT=ones1, rhs=battn, start=False, stop=True)
            expt = blk.tile([128, 128], F32, name="expt")
            nc.scalar.activation(out=expt, in_=ps_at, func=AF.Exp)
            den = blk.tile([128, M], F32, name="den")
            nc.vector.tensor_reduce(out=den, in_=expt[:].rearrange("p (m k) -> p m k", m=M),
                                    axis=AX.X, op=OP.add)
            nc.vector.reciprocal(out=den, in_=den)
            aw = blk.tile([128, 128], F32, name="aw")
            nc.vector.tensor_tensor(out=aw[:].rearrange("p (m k) -> p m k", m=M),
                                    in0=expt[:].rearrange("p (m k) -> p m k", m=M),
                                    in1=den[:, :, None].broadcast_to([128, M, LV * P]),
                                    op=OP.mult)

            # ---- sampling coordinates ----
            # refe[l, xy] = ref * (W|H) - 0.5  (small [128, 8] ops)
            refe = blk.tile([128, LV * 2], F32, name="refe")
            nc.vector.tensor_tensor(out=refe, in0=ref_t[:].rearrange("p l x -> p (l x)"),
                                    in1=refdims, op=OP.mult)
            nc.vector.tensor_scalar(out=refe, in0=refe, scalar1=0.5, scalar2=None,
                                    op0=OP.subtract)
            # expand refe (l,xy) -> (l,p,xy), then x = off + refe32 (bcast m)
            refe32 = blk.tile([128, LV * P * 2], F32, name="refe32")
            nc.vector.tensor_copy(
                out=refe32[:].rearrange("p (l q y) -> p l q y", l=LV, q=P),
                in_=refe[:].rearrange("p (l y) -> p l y", l=LV)[:, :, None, :]
                    .broadcast_to([128, LV, P, 2]))
            x = blk.tile([128, 256], F32, name="x")
            nc.vector.tensor_tensor(
                out=x[:].rearrange("p (m k) -> p m k", m=M),
                in0=ps_off[:].rearrange("p (m k) -> p m k", m=M),
                in1=refe32[:, None, :].broadcast_to([128, M, LV * P * 2]),
                op=OP.add)
            # clip to [-1, dim]
            nc.vector.tensor_scalar(out=x, in0=x, scalar1=-1.0, scalar2=None, op0=OP.max)
            nc.vector.tensor_tensor(out=x, in0=x, in1=dims, op=OP.min)

            xv = x[:].rearrange("p (k y) -> p k y", y=2)
            xc = xv[:, :, 0]   # [128, 128] strided views
            yc = xv[:, :, 1]

            # xQ = round(K * xc)
            xQ = blk.tile([128, 128], F32, name="xQ")
            nc.vector.tensor_scalar(out=xQ, in0=xc, scalar1=float(K), scalar2=TWO23,
                                    op0=OP.mult, op1=OP.add)
            nc.vector.tensor_scalar(out=xQ, in0=xQ, scalar1=TWO23, scalar2=None,
                                    op0=OP.subtract)
            # x0 = floor(xQ / K) = round(xQ/K - (K-1)/(2K)), tie-free
            x0 = blk.tile([128, 128], F32, name="x0")
            nc.vector.tensor_scalar(out=x0, in0=xQ, scalar1=1.0 / K,
                                    scalar2=(K - 1.0) / (2 * K),
                                    op0=OP.mult, op1=OP.subtract)
            nc.vector.tensor_scalar(out=x0, in0=x0, scalar1=TWO23, scalar2=TWO23,
                                    op0=OP.add, op1=OP.subtract)
            # y0 = clip(round(yc - 0.5), -1, H-1); wy = yc - y0
            y0 = blk.tile([128, 128], F32, name="y0")
            nc.vector.tensor_scalar(out=y0, in0=yc, scalar1=0.5, scalar2=TWO23,
                                    op0=OP.subtract, op1=OP.add)
            nc.vector.tensor_scalar(out=y0, in0=y0, scalar1=TWO23, scalar2=-1.0,
                                    op0=OP.subtract, op1=OP.max)
            nc.vector.tensor_tensor(out=y0, in0=y0, in1=dimm1y, op=OP.min)
            wy = blk.tile([128, 128], F32, name="wy")
            nc.vector.tensor_tensor(out=wy, in0=yc, in1=y0, op=OP.subtract)

            # sw[q, (m,l,p), y] : y0-weight = aw*(1-wy), y1-weight = aw*wy (bf16)
            sw = blk.tile([128, 256], BF16, name="sw")
            swv = sw[:].rearrange("p (k y) -> p k y", y=2)
            nc.vector.tensor_tensor(out=swv[:, :, 1], in0=aw, in1=wy, op=OP.mult)
            nc.vector.tensor_tensor(out=swv[:, :, 0], in0=aw, in1=swv[:, :, 1],
                                    op=OP.subtract)

            # idx = (s*M + m)*RG + LB_l + (x0+1)*(H+2) + (y0+1), s = xQ - K*x0
            s_t2 = blk.tile([128, 128], F32, name="s_t2")
            nc.vector.scalar_tensor_tensor(out=s_t2, in0=x0, scalar=-float(K), in1=xQ,
                                           op0=OP.mult, op1=OP.add)
            idxf = blk.tile([128, 128], F32, name="idxf")
            nc.vector.tensor_tensor(out=idxf, in0=x0, in1=h2t, op=OP.mult)
            nc.vector.tensor_tensor(out=idxf, in0=idxf, in1=y0, op=OP.add)
            nc.vector.tensor_tensor(out=idxf, in0=idxf, in1=cidx, op=OP.add)
            nc.vector.scalar_tensor_tensor(out=idxf, in0=s_t2, scalar=float(RG),
                                           in1=idxf, op0=OP.mult, op1=OP.add)
            idx16 = blk.tile([128, 128], I16, name="idx16")
            nc.vector.tensor_copy(out=idx16, in_=idxf)
            # wrap indices into dma_gather layout: fold2r[qlo, (m, pt, qhi)]
            fold1 = blk.tile([16, 8, 128], I16, name="fold1")
            for qhi in range(8):
                nc.sync.dma_start(out=fold1[:, qhi, :],
                                  in_=idx16[qhi * 16:(qhi + 1) * 16, :])
            fold2r = blk.tile([128, M * 128], I16, name="fold2r")
            nc.vector.tensor_copy(
                out=fold2r[0:16, :].rearrange("p (m k q) -> p m k q", m=M, k=16),
                in_=fold1[:].rearrange("p q (m k) -> p m k q", m=M))
            nc.sync.dma_start(out=fold2r[16:32, :], in_=fold2r[0:16, :])
            nc.sync.dma_start(out=fold2r[32:64, :], in_=fold2r[0:32, :])
            nc.sync.dma_start(out=fold2r[64:128, :], in_=fold2r[0:64, :])

            if DEBUG:
                nc.sync.dma_start(out=t["dbg_aw"][q0:q0 + 128, :], in_=aw)
                nc.sync.dma_start(out=t["dbg_idx"][q0:q0 + 128, :], in_=idxf)
                nc.sync.dma_start(out=t["dbg_x"][q0:q0 + 128, :], in_=x)
                dbg_sw = blk.tile([128, 256], F32, name="dbg_sw")
                nc.vector.tensor_copy(out=dbg_sw, in_=sw)
                nc.sync.dma_start(out=t["dbg_sw"][q0:q0 + 128, :], in_=dbg_sw)

            # ---- gather + blend per head ----
            sampled = blk.tile([128, DM], F32, name="sampled")
            for m in range(M):
                g2m = gpool.tile([128, 16, 2 * DH], F32, name="g2m")
                for hk in range(2):
                    nc.gpsimd.dma_gather(
                        out_ap=g2m[:, hk * 8:(hk + 1) * 8, :],
                        in_ap=G2v[:, m, :],
                        idxs_ap=fold2r[:, m * 128 + hk * 64:m * 128 + (hk + 1) * 64],
                        num_idxs=1024, num_idxs_reg=1024,
                        elem_size=2 * DH, elem_step=RB,
                        transpose=False, queue_num=0)
                wtm = gpool.tile([128, LV * P * 2 * DH], BF16, name="wtm")
                eng = nc.gpsimd if m in HEADS_ON_POOL else nc.vector
                eng.tensor_tensor(
                    out=wtm[:].rearrange("p (k y c) -> p k y c", y=2, c=DH),
                    in0=g2m[:].rearrange("p k (y c) -> p k y c", y=2),
                    in1=sw[:].rearrange("p (k y) -> p k y", y=2)[
                        :, m * 16:(m + 1) * 16, :, None]
                        .broadcast_to([128, 16, 2, DH]),
                    op=OP.mult)
                nc.vector.tensor_reduce(
                    out=sampled[:, m * DH:(m + 1) * DH],
                    in_=wtm[:].rearrange("p (u c) -> p c u", c=DH),
                    axis=AX.X, op=OP.add)

            if DEBUG:
                nc.sync.dma_start(out=t["dbg_sampled"][q0:q0 + 128, :], in_=sampled)

            # ---- output projection ----
            sT = []
            for c in range(2):
                tp = ps_t.tile([128, 128], F32, name="s_tp", tag="tp")
                nc.tensor.transpose(out=tp, in_=sampled[:, c * 128:(c + 1) * 128],
                                    identity=ident)
                sb = blk.tile([128, 128], BF16, name="sT")
                nc.scalar.activation(out=sb, in_=tp, func=AF.Copy)
                sT.append(sb)
            ps_h = ps_mm.tile([128, DM], F32, name="ps_h", tag="mm")
            for c in range(2):
                nc.tensor.matmul(out=ps_h, lhsT=sT[c], rhs=Wout[c], start=(c == 0), stop=False)
            nc.tensor.matmul(out=ps_h, lhsT=ones1, rhs=bout, start=False, stop=True)

            # ---- LN1 ----
            r1 = blk.tile([128, DM], F32, name="r1")
            nc.vector.tensor_tensor(out=r1, in0=qf_t, in1=ps_h, op=OP.add)
            h = _layernorm(nc, blk, r1, ln1g, ln1b, eps_t)

            # ---- FFN ----
            hT = []
            for c in range(2):
                tp = ps_t.tile([128, 128], F32, name="h_tp", tag="tp")
                nc.tensor.transpose(out=tp, in_=h[:, c * 128:(c + 1) * 128], identity=ident)
                sb = blk.tile([128, 128], BF16, name="hT")
                nc.scalar.activation(out=sb, in_=tp, func=AF.Copy)
                hT.append(sb)
            relu1 = []
            for fc in range(8):
                pf = ps_f1.tile([128, 128], F32, name="pf")
                for c in range(2):
                    nc.tensor.matmul(out=pf, lhsT=W1[c][:, fc * 128:(fc + 1) * 128],
                                     rhs=hT[c], start=(c == 0), stop=False)
                nc.tensor.matmul(out=pf, lhsT=b1[:, fc * 128:(fc + 1) * 128],
                                 rhs=ones1, start=False, stop=True)
                rt = blk.tile([128, 128], BF16, name=f"relu1_{fc}")
                nc.scalar.activation(out=rt, in_=pf, func=AF.Relu)
                relu1.append(rt)
            ps_o = ps_mm.tile([128, DM], F32, name="ps_o", tag="mm")
            for fc in range(8):
                nc.tensor.matmul(out=ps_o, lhsT=relu1[fc], rhs=W2[fc], start=(fc == 0),
                                 stop=False)
            nc.tensor.matmul(out=ps_o, lhsT=ones1, rhs=b2, start=False, stop=True)

            # ---- LN2 + store ----
            r2 = blk.tile([128, DM], F32, name="r2")
            nc.vector.tensor_tensor(out=r2, in0=h, in1=ps_o, op=OP.add)
            o = _layernorm(nc, blk, r2, ln2g, ln2b, eps_t)
            nc.sync.dma_start(out=out_ap[q0:q0 + 128, :], in_=o)


def _layernorm(nc, pool, r, g, b, eps_t):
    stats = pool.tile([128, 6], F32, name="ln_stats")
    nc.vector.bn_stats(out=stats, in_=r)
    mv = pool.tile([128, 2], F32, name="ln_mv")
    nc.vector.bn_aggr(out=mv, in_=stats)
    rstd = pool.tile([128, 1], F32, name="ln_rstd")
    nc.scalar.activation(out=rstd, in_=mv[:, 1:2], func=AF.Sqrt, bias=eps_t)
    nc.vector.reciprocal(out=rstd, in_=rstd)
    xs = pool.tile([128, DM], F32, name="ln_xs")
    nc.vector.tensor_scalar(out=xs, in0=r, scalar1=mv[:, 0:1], scalar2=rstd,
                            op0=OP.subtract, op1=OP.mult)
    h = pool.tile([128, DM], F32, name="ln_h")
    nc.vector.tensor_tensor(out=h, in0=xs, in1=g, op=OP.mult)
    nc.vector.tensor_tensor(out=h, in0=h, in1=b, op=OP.add)
    return h


# ---------------------------------------------------------------------------
# host side
# ---------------------------------------------------------------------------

_prog_cache = {}


def _get_program():
    if "nc" not in _prog_cache:
        _prog_cache["nc"] = build_program()
    return _prog_cache["nc"]


def _host_constants():
    f = np.float32
    H = np.array([h for h, w in SHAPES], np.int64)
    W = np.array([w for h, w in SHAPES], np.int64)
    # refdims [(l, xy)]: xy=0 -> W, xy=1 -> H
    refd = np.zeros((LV, 2), f)
    refd[:, 0] = W; refd[:, 1] = H
    refdims = np.broadcast_to(refd.reshape(1, -1), (128, LV * 2)).copy()
    # dims [(m,l,p,xy)] -> W | H (clip max)
    dm = np.zeros((M, LV, P, 2), f)
    dm[:, :, :, 0] = W[None, :, None]
    dm[:, :, :, 1] = H[None, :, None]
    dims = np.broadcast_to(dm.reshape(1, -1), (128, M * LV * P * 2)).copy()
    # dimm1y [(m,l,p)] -> H - 1
    d1 = np.zeros((M, LV, P), f)
    d1[:, :, :] = (H - 1)[None, :, None]
    dimm1y = np.broadcast_to(d1.reshape(1, -1), (128, M * LV * P)).copy()
    # h2t [(m,l,p)] -> H + 2 (column-major x-stride)
    h2 = np.zeros((M, LV, P), f)
    h2[:, :, :] = (H + 2)[None, :, None]
    h2t = np.broadcast_to(h2.reshape(1, -1), (128, M * LV * P)).copy()
    # cidx [(m,l,p)] -> LB_l + (H_l+2) + 1 + 1 (one leading G2 pad row)
    ci = np.zeros((M, LV, P), f)
    for m in range(M):
        for li in range(LV):
            ci[m, li, :] = LB[li] + (H[li] + 2) + 2
    cidx = np.broadcast_to(ci.reshape(1, -1), (128, M * LV * P)).copy()
    return refdims, dims, dimm1y, h2t, cidx


def _build_in_maps(inputs):
    src = np.asarray(inputs["src"], np.float32)
    q_feat = np.asarray(inputs["q_feat"], np.float32)
    q_pos = np.asarray(inputs["q_pos"], np.float32)
    ref = np.asarray(inputs["reference_points"], np.float32)
    ss = np.asarray(inputs["spatial_shapes"])
    lsi_in = np.asarray(inputs["level_start_index"])
    assert src.shape == (B, L, DM), src.shape
    assert [tuple(r) for r in ss.tolist()] == SHAPES, ss
    assert lsi_in.tolist() == [0, 8464, 10580, 11109], lsi_in

    refdims, dims, dimm1y, h2t, cidx = _host_constants()

    def as_bf16(a):
        import ml_dtypes
        return np.asarray(a, np.float32).astype(ml_dtypes.bfloat16)

    common = {
        "Woff": as_bf16(inputs["W_off"]),
        "Wattn": as_bf16(inputs["W_attn"]),
        "Wv": as_bf16(inputs["W_v"]),
        "Wout": as_bf16(inputs["W_out"]),
        "W1": as_bf16(inputs["W1"]),
        "W2": as_bf16(inputs["W2"]),
        "boff": as_bf16(inputs["b_off"]).reshape(1, -1),
        "battn": as_bf16(inputs["b_attn"]).reshape(1, -1),
        "bv": as_bf16(inputs["b_v"]).reshape(1, -1),
        "bout": as_bf16(inputs["b_out"]).reshape(1, -1),
        "b1": as_bf16(inputs["b1"]).reshape(1, -1),
        "b2": as_bf16(inputs["b2"]).reshape(1, -1),
        "ln1g": np.broadcast_to(np.asarray(inputs["ln1_g"], np.float32), (128, DM)).copy(),
        "ln1b": np.broadcast_to(np.asarray(inputs["ln1_b"], np.float32), (128, DM)).copy(),
        "ln2g": np.broadcast_to(np.asarray(inputs["ln2_g"], np.float32), (128, DM)).copy(),
        "ln2b": np.broadcast_to(np.asarray(inputs["ln2_b"], np.float32), (128, DM)).copy(),
        "ident": np.eye(128, dtype=np.float32),
        "ones1": as_bf16(np.ones((1, 128), np.float32)),
        "refdims": refdims, "dims": dims, "dimm1y": dimm1y, "h2t": h2t, "cidx": cidx,
    }

    halves = [(0, LC), (LC, L - LC)]
    in_maps = []
    for core in range(8):
        b = core // 2
        h0, hn = halves[core % 2]
        qf = np.zeros((LCPAD, DM), np.float32)
        qp = np.zeros((LCPAD, DM), np.float32)
        rf = np.zeros((LCPAD, LV, 2), np.float32)
        qf[:hn] = q_feat[b, h0:h0 + hn]
        qp[:hn] = q_pos[b, h0:h0 + hn]
        rf[:hn] = ref[b, h0:h0 + hn]
        m = dict(common)
        m.update({"qf": qf, "qp": qp, "ref": rf, "srcb": src[b]})
        in_maps.append(m)
    return in_maps


def kernel(**inputs):
    from concourse.bass_utils import run_bass_kernel_spmd

    in_maps = _build_in_maps(inputs)
    nc = _get_program()
    res = run_bass_kernel_spmd(nc, in_maps, core_ids=list(range(8)))

    halves = [(0, LC), (LC, L - LC)]
    out = np.zeros((B, L, DM), np.float32)
    for core in range(8):
        b = core // 2
        h0, hn = halves[core % 2]
        out[b, h0:h0 + hn] = res.results[core]["out"][:hn]
    return out


# revision 26
# speedup vs baseline: 1.6139x; 1.4453x over previous
"""Deformable Transformer Encoder Layer — Trainium2 Bass kernel (v2).

Sharding: 8 cores = (batch b in 0..3) x (query-half h in 0..1).
Each core computes the full layer for its (b, query-half) slice.

Sampling strategy (per core):
  - value projection over the FULL batch image (PE), stored bf16.
  - zero-PADDED per-level grid v1p ((H+2)x(W+2) rows per level) makes
    grid_sample zero-padding semantics exact with plain clip+floor math
    (no OOB masks).
  - K=4 x-pre-lerped grids G_s = (1-s/4) v1p + (s/4) shift_x(v1p),
    laid out [s][m][row][32ch] bf16, so one sample point needs a single
    contiguous 128B run (y-row pair) fetched via gpsimd indirect DMA.
    x-fraction is quantized to 1/4 pixel (error ~1e-3 of value scale).
  - blend: per head, multiply gathered pairs by combined
    attention*(1-wy, wy) weights (Pool engine) and a segmented reduce
    (DVE), followed by out-proj / LN / FFN / LN as in the baseline.
No cross-core communication; host reassembles the output.
"""

import os
import sys
import numpy as np

for _p in ("/opt/trn_rl_repo", "/root/.axon_site/_ro/trn_rl_repo"):
    if os.path.isdir(_p) and _p not in sys.path:
        sys.path.insert(0, _p)

import concourse.bass as bass
import concourse.mybir as mybir
import concourse.tile as tile
from concourse import bacc
from concourse.bass import AP

F32 = mybir.dt.float32
BF16 = mybir.dt.bfloat16
I32 = mybir.dt.int32
I16 = mybir.dt.int16
AF = mybir.ActivationFunctionType
OP = mybir.AluOpType
AX = mybir.AxisListType

# Problem constants (checked against inputs at runtime on host).
M, LV, P, DM, DH, DF = 8, 4, 4, 256, 32, 1024
L = 11253
B = 4
LC = 5627           # queries per core (split [5627, 5626])
LCPAD = 5632        # 44 * 128
NBLK = LCPAD // 128
EPS = 1e-5
TWO23 = 12582912.0  # 3*2^22: rounding shift
SHAPES = [(92, 92), (46, 46), (23, 23), (12, 12)]
K = 2               # x-lerp quantization levels (idx must fit int16)
RG = 12288          # padded-grid rows per (s, m), multiple of 1024
R1 = sum((h + 2) * (w + 2) for h, w in SHAPES)  # 11961 real padded rows
NT = RG // 1024     # macro-tiles in grid build
LB = []             # level base rows (padded space)
_acc = 0
for _h, _w in SHAPES:
    LB.append(_acc)
    _acc += (_h + 2) * (_w + 2)

DEBUG = False
# heads whose blend-multiply runs on the Pool (gpsimd) engine
HEADS_ON_POOL = (0, 1, 2, 3)


def build_program():
    nc = bacc.Bacc("TRN2", target_bir_lowering=False, debug=False, enable_asserts=False)

    t = {}
    def inp(name, shape, dtype=F32):
        t[name] = nc.dram_tensor(name, list(shape), dtype, kind="ExternalInput").ap()
        return t[name]

    # per-core data
    inp("qf", (LCPAD, DM)); inp("qp", (LCPAD, DM)); inp("ref", (LCPAD, LV, 2))
    inp("srcb", (L, DM))
    # weights (bf16 on host for matmul rhs)
    inp("Woff", (DM, M * LV * P * 2), BF16)
    inp("Wattn", (DM, M * LV * P), BF16)
    inp("Wv", (DM, DM), BF16)
    inp("Wout", (DM, DM), BF16)
    inp("W1", (DM, DF), BF16)
    inp("W2", (DF, DM), BF16)
    # biases as [1, N] rows (rank-1 matmul trick), bf16
    inp("boff", (1, M * LV * P * 2), BF16)
    inp("battn", (1, M * LV * P), BF16)
    inp("bv", (1, DM), BF16)
    inp("bout", (1, DM), BF16)
    inp("b1", (1, DF), BF16)
    inp("b2", (1, DM), BF16)
    # layernorm params replicated across partitions (f32)
    inp("ln1g", (128, DM)); inp("ln1b", (128, DM))
    inp("ln2g", (128, DM)); inp("ln2b", (128, DM))
    # constants
    inp("ident", (128, 128))              # f32 identity for PE transpose
    inp("ones1", (1, 128), BF16)          # rank-1 lhsT of ones
    inp("refdims", (128, LV * 2))         # (l,xy) -> W_l | H_l, replicated
    inp("dims", (128, M * LV * P * 2))    # (m,l,p,xy) -> W_l | H_l
    inp("dimm1y", (128, M * LV * P))      # (m,l,p) -> H_l - 1
    inp("h2t", (128, M * LV * P))         # (m,l,p) -> H_l + 2 (col-major x stride)
    inp("cidx", (128, M * LV * P))        # (m,l,p) -> m*RG + LB_l + (H_l+2) + 1

    out_ap = nc.dram_tensor("out", [LCPAD, DM], F32, kind="ExternalOutput").ap()
    if DEBUG:
        for nm, w in (("dbg_sampled", 256), ("dbg_aw", 128), ("dbg_idx", 128),
                      ("dbg_sw", 256), ("dbg_x", 256)):
            t[nm] = nc.dram_tensor(nm, [LCPAD, w], F32, kind="ExternalOutput").ap()

    with tile.TileContext(nc) as tc:
        _build(tc, out_ap, t)

    nc.compile()
    return nc


def _build(tc, out_ap, t):
    nc = tc.nc
    from contextlib import ExitStack
    ctx = ExitStack()
    with ctx:
        consts = ctx.enter_context(tc.tile_pool(name="consts", bufs=1))
        wpool = ctx.enter_context(tc.tile_pool(name="wpool", bufs=1))
        vblk = ctx.enter_context(tc.tile_pool(name="vblk", bufs=3))
        gblk = ctx.enter_context(tc.tile_pool(name="gblk", bufs=2))
        blk = ctx.enter_context(tc.tile_pool(name="blk", bufs=3))
        gpoolA = ctx.enter_context(tc.tile_pool(name="gpoolA", bufs=16))
        gpoolB = ctx.enter_context(tc.tile_pool(name="gpoolB", bufs=3))
        ps_t = ctx.enter_context(tc.tile_pool(name="ps_t", bufs=2, space="PSUM"))
        ps_mm = ctx.enter_context(tc.tile_pool(name="ps_mm", bufs=4, space="PSUM"))
        ps_f1 = ctx.enter_context(tc.tile_pool(name="ps_f1", bufs=2, space="PSUM"))
        dram = ctx.enter_context(tc.tile_pool(name="dram", bufs=1, space="DRAM"))

        # ---- resident constants / weights in SBUF ----
        def ld(name):
            ap = t[name]
            tile_ = consts.tile(list(ap.shape), ap.dtype, name=name + "_s")
            nc.sync.dma_start(out=tile_, in_=ap)
            return tile_

        ident = ld("ident")
        ones1 = ld("ones1")
        eps_t = consts.tile([128, 1], F32, name="eps_t")
        nc.vector.memset(eps_t, EPS)
        refdims = ld("refdims"); dims = ld("dims")
        dimm1y = ld("dimm1y"); h2t = ld("h2t"); cidx = ld("cidx")
        ln1g = ld("ln1g"); ln1b = ld("ln1b"); ln2g = ld("ln2g"); ln2b = ld("ln2b")
        boff = ld("boff"); battn = ld("battn"); bv = ld("bv")
        bout = ld("bout"); b1 = ld("b1"); b2 = ld("b2")

        def ldw(name, kchunks):
            ap = t[name]
            K_, N = ap.shape
            tiles = []
            for k in range(kchunks):
                w_ = wpool.tile([128, N], ap.dtype, name=f"{name}_{k}")
                nc.sync.dma_start(out=w_, in_=ap[k * 128:(k + 1) * 128, :])
                tiles.append(w_)
            return tiles

        Woff = ldw("Woff", 2); Wattn = ldw("Wattn", 2); Wv = ldw("Wv", 2)
        Wout = ldw("Wout", 2); W1 = ldw("W1", 2); W2 = ldw("W2", 8)

        # ---------------------------------------------------------------
        # Phase 1: value projection -> value1 (bf16, [L, M*DH]) in DRAM
        # ---------------------------------------------------------------
        value1 = dram.tile([L, DM], BF16, name="value1")

        for vb in range((L + 127) // 128):
            p0 = vb * 128
            pn = min(128, L - p0)
            s_t = vblk.tile([128, DM], F32, name="s_t")
            nc.sync.dma_start(out=s_t[:pn], in_=t["srcb"][p0:p0 + pn, :])
            sT = []
            for c in range(2):
                tp = ps_t.tile([128, 128], F32, name="v_tp", tag="tp")
                nc.tensor.transpose(out=tp[:, :pn], in_=s_t[:pn, c * 128:(c + 1) * 128],
                                    identity=ident[:pn, :pn])
                sb = vblk.tile([128, 128], BF16, name="v_sT")
                nc.scalar.activation(out=sb[:, :pn], in_=tp[:, :pn], func=AF.Copy)
                sT.append(sb)
            pv = ps_mm.tile([128, DM], F32, name="v_ps", tag="mm")
            for c in range(2):
                nc.tensor.matmul(out=pv[:pn], lhsT=sT[c][:, :pn], rhs=Wv[c],
                                 start=(c == 0), stop=False)
            nc.tensor.matmul(out=pv[:pn], lhsT=ones1[:, :pn], rhs=bv, start=False, stop=True)
            vt = vblk.tile([128, DM], BF16, name="v_out")
            nc.scalar.activation(out=vt[:pn], in_=pv[:pn], func=AF.Copy)
            nc.sync.dma_start(out=value1[p0:p0 + pn, :], in_=vt[:pn])

        # ---------------------------------------------------------------
        # Phase 2: padded grids (COLUMN-major per level: row = x'*(H+2)+y')
        # so that consecutive grid rows are y-neighbors (gathered as one
        # 128B run), then K x-lerped grids G.
        # ---------------------------------------------------------------
        v1p = dram.tile([RG, DM], BF16, name="v1p")
        v1px = dram.tile([RG, DM], BF16, name="v1px")  # v1p shifted by one x
        zt = vblk.tile([128, DM], BF16, name="zt")
        nc.vector.memset(zt, 0.0)
        for zb in range(RG // 128):
            p0 = zb * 128
            nc.sync.dma_start(out=v1p[p0:p0 + 128, :], in_=zt)
        # v1px tails per level are unwritten; those rows only feed G2 rows
        # that are never gathered, but zero the last level tail for safety
        nc.sync.dma_start(out=v1px[RG - 128:RG, :], in_=zt)

        # interior copies per level:
        # v1p[LB + (x+1)*(H+2) + (y+1)] = value1[lsi + y*W + x]
        lsi = 0
        for li, (H, W) in enumerate(SHAPES):
            src = value1[:].rearrange("r c -> (r c)")[
                lsi * DM:(lsi + H * W) * DM].rearrange("(y x c) -> y x c", y=H, x=W)
            _d0 = (LB[li] + (H + 2) + 1) * DM
            dst = v1p[:].rearrange("r c -> (r c)")[
                _d0:_d0 + W * (H + 2) * DM].rearrange(
                    "(x q) -> x q", x=W)[:, :H * DM].rearrange(
                    "x (y c) -> y x c", y=H)
            nc.sync.dma_start(out=dst, in_=src)
            lsi += H * W

        # v1px[r] = v1p[r + (H_l+2)] within each level block (x-shift)
        for li, (H, W) in enumerate(SHAPES):
            bs = (H + 2) * (W + 2)
            sh = H + 2
            src = v1p[:].rearrange("r c -> (r c)")[
                (LB[li] + sh) * DM:(LB[li] + bs) * DM]
            dst = v1px[:].rearrange("r c -> (r c)")[
                LB[li] * DM:(LB[li] + bs - sh) * DM]
            nc.sync.dma_start(out=dst, in_=src)

        # G2: K x-lerped, y-pair-duplicated grids, f32, one leading pad row.
        # Row 1 + s*RG + r holds, per head, [lerp_s(r), lerp_s(r+1)]
        # (column-major => r+1 is the y-neighbor). A sample point is one
        # 256B run: G2[1 + s*RG + row, m, :, :].
        RB = M * 2 * DH  # row elems (512)
        G2 = dram.tile([K * RG + 1, RB], F32, name="G2")
        g2f = G2[:].rearrange("r c -> (r c)")
        for it in range(NT):
            r0 = it * 1024
            t0 = gblk.tile([128, 8 * DM], BF16, name="g_t0")
            t1 = gblk.tile([128, 8 * DM], BF16, name="g_t1")
            nc.sync.dma_start(
                out=t0, in_=v1p[r0:r0 + 1024, :].rearrange("(p j) c -> p (j c)", p=128))
            nc.sync.dma_start(
                out=t1, in_=v1px[r0:r0 + 1024, :].rearrange("(p j) c -> p (j c)", p=128))
            d = gblk.tile([128, 8 * DM], BF16, name="g_d")
            nc.vector.tensor_tensor(out=d, in0=t1, in1=t0, op=OP.subtract)
            da = gblk.tile([128, 8 * DM], BF16, name="g_da")
            for s in range(K):
                gs = gblk.tile([128, 8 * DM], F32, name="g_gs")
                if s == 0:
                    nc.vector.tensor_copy(out=gs, in_=t0)
                else:
                    nc.vector.tensor_scalar(out=da, in0=d, scalar1=float(s) / K,
                                            scalar2=None, op0=OP.mult)
                    nc.vector.tensor_tensor(out=gs, in0=da, in1=t0, op=OP.add)
                # gs row (p*8+j) -> G2[1 + s*RG + r0 + p*8+j - dy][m][dy][:]
                # split into 16-partition chunks (1024 descriptors each)
                for dy in range(2):
                    for jc in range(4):
                        src = gs[jc * 32:(jc + 1) * 32, :].rearrange(
                            "p (j m c) -> p j m c", m=M, j=8)
                        off = (1 + s * RG + r0 + jc * 256 - dy) * RB + dy * DH
                        dst = g2f[off:off + 256 * RB].rearrange(
                            "(p j q) -> p j q", p=32, j=8).rearrange(
                            "p j (m y c) -> p j m y c", m=M, y=2)[:, :, :, 0, :]
                        nc.sync.dma_start(out=dst, in_=src)

        G2v = G2[:].rearrange("r (m q) -> r m q", m=M)  # [K*RG+1, M, 2*DH]

        # ---------------------------------------------------------------
        # Phase 3: main per-block loop, software-pipelined in two stages:
        #   A(ib): loads, projections, softmax, coords, idx, folds, gathers
        #   B(ib): blend (mult+reduce), out-proj, LN1, FFN, LN2, store
        # Emission order A(0), A(1), B(0), A(2), B(1), ... hides the
        # gather/blend latency of block i behind stage A of block i+1.
        # ---------------------------------------------------------------
        def stageA(ib):
            st = {}
            q0 = ib * 128
            qf_t = blk.tile([128, DM], F32, name="qf_t")
            qp_t = blk.tile([128, DM], F32, name="qp_t")
            ref_t = blk.tile([128, LV, 2], F32, name="ref_t")
            nc.sync.dma_start(out=qf_t, in_=t["qf"][q0:q0 + 128, :])
            nc.sync.dma_start(out=qp_t, in_=t["qp"][q0:q0 + 128, :])
            nc.sync.dma_start(out=ref_t, in_=t["ref"][q0:q0 + 128, :, :])
            st["qf_t"] = qf_t

            qT = []
            for src_t in (qf_t, qp_t):
                for c in range(2):
                    tp = ps_t.tile([128, 128], F32, name="q_tp", tag="tp")
                    nc.tensor.transpose(out=tp, in_=src_t[:, c * 128:(c + 1) * 128],
                                        identity=ident)
                    sb = blk.tile([128, 128], BF16, name="qT")
                    nc.scalar.activation(out=sb, in_=tp, func=AF.Copy)
                    qT.append(sb)

            ps_off = ps_mm.tile([128, 256], F32, name="ps_off", tag="mm")
            for i, w_ in ((0, Woff[0]), (1, Woff[1]), (2, Woff[0]), (3, Woff[1])):
                nc.tensor.matmul(out=ps_off, lhsT=qT[i], rhs=w_, start=(i == 0), stop=False)
            nc.tensor.matmul(out=ps_off, lhsT=ones1, rhs=boff, start=False, stop=True)

            ps_at = ps_mm.tile([128, 128], F32, name="ps_at", tag="mm")
            for i, w_ in ((0, Wattn[0]), (1, Wattn[1]), (2, Wattn[0]), (3, Wattn[1])):
                nc.tensor.matmul(out=ps_at, lhsT=qT[i], rhs=w_, start=(i == 0), stop=False)
            nc.tensor.matmul(out=ps_at, lhsT=ones1, rhs=battn, start=False, stop=True)
            expt = blk.tile([128, 128], F32, name="expt")
            nc.scalar.activation(out=expt, in_=ps_at, func=AF.Exp)
            den = blk.tile([128, M], F32, name="den")
            nc.vector.tensor_reduce(out=den, in_=expt[:].rearrange("p (m k) -> p m k", m=M),
                                    axis=AX.X, op=OP.add)
            nc.vector.reciprocal(out=den, in_=den)
            aw = blk.tile([128, 128], F32, name="aw")
            nc.vector.tensor_tensor(out=aw[:].rearrange("p (m k) -> p m k", m=M),
                                    in0=expt[:].rearrange("p (m k) -> p m k", m=M),
                                    in1=den[:, :, None].broadcast_to([128, M, LV * P]),
                                    op=OP.mult)

            # ---- sampling coordinates ----
            refe = blk.tile([128, LV * 2], F32, name="refe")
            nc.vector.tensor_tensor(out=refe, in0=ref_t[:].rearrange("p l x -> p (l x)"),
                                    in1=refdims, op=OP.mult)
            nc.vector.tensor_scalar(out=refe, in0=refe, scalar1=0.5, scalar2=None,
                                    op0=OP.subtract)
            refe32 = blk.tile([128, LV * P * 2], F32, name="refe32")
            nc.vector.tensor_copy(
                out=refe32[:].rearrange("p (l q y) -> p l q y", l=LV, q=P),
                in_=refe[:].rearrange("p (l y) -> p l y", l=LV)[:, :, None, :]
                    .broadcast_to([128, LV, P, 2]))
            x = blk.tile([128, 256], F32, name="x")
            nc.vector.tensor_tensor(
                out=x[:].rearrange("p (m k) -> p m k", m=M),
                in0=ps_off[:].rearrange("p (m k) -> p m k", m=M),
                in1=refe32[:, None, :].broadcast_to([128, M, LV * P * 2]),
                op=OP.add)
            nc.vector.tensor_scalar(out=x, in0=x, scalar1=-1.0, scalar2=None, op0=OP.max)
            nc.vector.tensor_tensor(out=x, in0=x, in1=dims, op=OP.min)

            xv = x[:].rearrange("p (k y) -> p k y", y=2)
            xc = xv[:, :, 0]
            yc = xv[:, :, 1]

            xQ = blk.tile([128, 128], F32, name="xQ")
            nc.vector.tensor_scalar(out=xQ, in0=xc, scalar1=float(K), scalar2=TWO23,
                                    op0=OP.mult, op1=OP.add)
            nc.vector.tensor_scalar(out=xQ, in0=xQ, scalar1=TWO23, scalar2=None,
                                    op0=OP.subtract)
            x0 = blk.tile([128, 128], F32, name="x0")
            nc.vector.tensor_scalar(out=x0, in0=xQ, scalar1=1.0 / K,
                                    scalar2=(K - 1.0) / (2 * K),
                                    op0=OP.mult, op1=OP.subtract)
            nc.vector.tensor_scalar(out=x0, in0=x0, scalar1=TWO23, scalar2=TWO23,
                                    op0=OP.add, op1=OP.subtract)
            y0 = blk.tile([128, 128], F32, name="y0")
            nc.vector.tensor_scalar(out=y0, in0=yc, scalar1=0.5, scalar2=TWO23,
                                    op0=OP.subtract, op1=OP.add)
            nc.vector.tensor_scalar(out=y0, in0=y0, scalar1=TWO23, scalar2=-1.0,
                                    op0=OP.subtract, op1=OP.max)
            nc.vector.tensor_tensor(out=y0, in0=y0, in1=dimm1y, op=OP.min)
            wy = blk.tile([128, 128], F32, name="wy")
            nc.vector.tensor_tensor(out=wy, in0=yc, in1=y0, op=OP.subtract)

            sw = blk.tile([128, 256], BF16, name="sw")
            swv = sw[:].rearrange("p (k y) -> p k y", y=2)
            nc.vector.tensor_tensor(out=swv[:, :, 1], in0=aw, in1=wy, op=OP.mult)
            nc.vector.tensor_tensor(out=swv[:, :, 0], in0=aw, in1=swv[:, :, 1],
                                    op=OP.subtract)
            st["sw"] = sw

            s_t2 = blk.tile([128, 128], F32, name="s_t2")
            nc.vector.scalar_tensor_tensor(out=s_t2, in0=x0, scalar=-float(K), in1=xQ,
                                           op0=OP.mult, op1=OP.add)
            idxf = blk.tile([128, 128], F32, name="idxf")
            nc.vector.tensor_tensor(out=idxf, in0=x0, in1=h2t, op=OP.mult)
            nc.vector.tensor_tensor(out=idxf, in0=idxf, in1=y0, op=OP.add)
            nc.vector.tensor_tensor(out=idxf, in0=idxf, in1=cidx, op=OP.add)
            nc.vector.scalar_tensor_tensor(out=idxf, in0=s_t2, scalar=float(RG),
                                           in1=idxf, op0=OP.mult, op1=OP.add)
            idx16 = blk.tile([128, 128], I16, name="idx16")
            nc.vector.tensor_copy(out=idx16, in_=idxf)
            fold1 = blk.tile([16, 8, 128], I16, name="fold1")
            for qhi in range(8):
                nc.sync.dma_start(out=fold1[:, qhi, :],
                                  in_=idx16[qhi * 16:(qhi + 1) * 16, :])
            fold2r = blk.tile([128, M * 128], I16, name="fold2r")
            nc.vector.tensor_copy(
                out=fold2r[0:16, :].rearrange("p (m k q) -> p m k q", m=M, k=16),
                in_=fold1[:].rearrange("p q (m k) -> p m k q", m=M))
            nc.sync.dma_start(out=fold2r[16:32, :], in_=fold2r[0:16, :])
            nc.sync.dma_start(out=fold2r[32:64, :], in_=fold2r[0:32, :])
            nc.sync.dma_start(out=fold2r[64:128, :], in_=fold2r[0:64, :])

            if DEBUG:
                nc.sync.dma_start(out=t["dbg_aw"][q0:q0 + 128, :], in_=aw)
                nc.sync.dma_start(out=t["dbg_idx"][q0:q0 + 128, :], in_=idxf)
                nc.sync.dma_start(out=t["dbg_x"][q0:q0 + 128, :], in_=x)
                dbg_sw = blk.tile([128, 256], F32, name="dbg_sw")
                nc.vector.tensor_copy(out=dbg_sw, in_=sw)
                nc.sync.dma_start(out=t["dbg_sw"][q0:q0 + 128, :], in_=dbg_sw)

            st["g2m"] = []
            for m in range(M):
                g2m = gpoolA.tile([128, 16, 2 * DH], F32, name="g2m")
                for hk in range(2):
                    nc.gpsimd.dma_gather(
                        out_ap=g2m[:, hk * 8:(hk + 1) * 8, :],
                        in_ap=G2v[:, m, :],
                        idxs_ap=fold2r[:, m * 128 + hk * 64:m * 128 + (hk + 1) * 64],
                        num_idxs=1024, num_idxs_reg=1024,
                        elem_size=2 * DH, elem_step=RB,
                        transpose=False, queue_num=0)
                st["g2m"].append(g2m)
            return st

        def stageB(ib, st):
            q0 = ib * 128
            sw = st["sw"]
            sampled = blk.tile([128, DM], F32, name="sampled")
            for m in range(M):
                g2m = st["g2m"][m]
                wtm = gpoolB.tile([128, LV * P * 2 * DH], BF16, name="wtm")
                eng = nc.gpsimd if m in HEADS_ON_POOL else nc.vector
                eng.tensor_tensor(
                    out=wtm[:].rearrange("p (k y c) -> p k y c", y=2, c=DH),
                    in0=g2m[:].rearrange("p k (y c) -> p k y c", y=2),
                    in1=sw[:].rearrange("p (k y) -> p k y", y=2)[
                        :, m * 16:(m + 1) * 16, :, None]
                        .broadcast_to([128, 16, 2, DH]),
                    op=OP.mult)
                nc.vector.tensor_reduce(
                    out=sampled[:, m * DH:(m + 1) * DH],
                    in_=wtm[:].rearrange("p (u c) -> p c u", c=DH),
                    axis=AX.X, op=OP.add)

            if DEBUG:
                nc.sync.dma_start(out=t["dbg_sampled"][q0:q0 + 128, :], in_=sampled)

            sT = []
            for c in range(2):
                tp = ps_t.tile([128, 128], F32, name="s_tp", tag="tp")
                nc.tensor.transpose(out=tp, in_=sampled[:, c * 128:(c + 1) * 128],
                                    identity=ident)
                sb = blk.tile([128, 128], BF16, name="sT")
                nc.scalar.activation(out=sb, in_=tp, func=AF.Copy)
                sT.append(sb)
            ps_h = ps_mm.tile([128, DM], F32, name="ps_h", tag="mm")
            for c in range(2):
                nc.tensor.matmul(out=ps_h, lhsT=sT[c], rhs=Wout[c], start=(c == 0), stop=False)
            nc.tensor.matmul(out=ps_h, lhsT=ones1, rhs=bout, start=False, stop=True)

            r1 = blk.tile([128, DM], F32, name="r1")
            nc.vector.tensor_tensor(out=r1, in0=st["qf_t"], in1=ps_h, op=OP.add)
            h = _layernorm(nc, blk, r1, ln1g, ln1b, eps_t)

            hT = []
            for c in range(2):
                tp = ps_t.tile([128, 128], F32, name="h_tp", tag="tp")
                nc.tensor.transpose(out=tp, in_=h[:, c * 128:(c + 1) * 128], identity=ident)
                sb = blk.tile([128, 128], BF16, name="hT")
                nc.scalar.activation(out=sb, in_=tp, func=AF.Copy)
                hT.append(sb)
            relu1 = []
            for fc in range(8):
                pf = ps_f1.tile([128, 128], F32, name="pf")
                for c in range(2):
                    nc.tensor.matmul(out=pf, lhsT=W1[c][:, fc * 128:(fc + 1) * 128],
                                     rhs=hT[c], start=(c == 0), stop=False)
                nc.tensor.matmul(out=pf, lhsT=b1[:, fc * 128:(fc + 1) * 128],
                                 rhs=ones1, start=False, stop=True)
                rt = blk.tile([128, 128], BF16, name=f"relu1_{fc}")
                nc.scalar.activation(out=rt, in_=pf, func=AF.Relu)
                relu1.append(rt)
            ps_o = ps_mm.tile([128, DM], F32, name="ps_o", tag="mm")
            for fc in range(8):
                nc.tensor.matmul(out=ps_o, lhsT=relu1[fc], rhs=W2[fc], start=(fc == 0),
                                 stop=False)
            nc.tensor.matmul(out=ps_o, lhsT=ones1, rhs=b2, start=False, stop=True)

            r2 = blk.tile([128, DM], F32, name="r2")
            nc.vector.tensor_tensor(out=r2, in0=h, in1=ps_o, op=OP.add)
            o = _layernorm(nc, blk, r2, ln2g, ln2b, eps_t)
            nc.sync.dma_start(out=out_ap[q0:q0 + 128, :], in_=o)

        prev = None
        for ib in range(NBLK):
            st = stageA(ib)
            if prev is not None:
                stageB(ib - 1, prev)
            prev = st
        stageB(NBLK - 1, prev)


def _layernorm(nc, pool, r, g, b, eps_t):
    stats = pool.tile([128, 6], F32, name="ln_stats")
    nc.vector.bn_stats(out=stats, in_=r)
    mv = pool.tile([128, 2], F32, name="ln_mv")
    nc.vector.bn_aggr(out=mv, in_=stats)
    rstd = pool.tile([128, 1], F32, name="ln_rstd")
    nc.scalar.activation(out=rstd, in_=mv[:, 1:2], func=AF.Sqrt, bias=eps_t)
    nc.vector.reciprocal(out=rstd, in_=rstd)
    xs = pool.tile([128, DM], F32, name="ln_xs")
    nc.vector.tensor_scalar(out=xs, in0=r, scalar1=mv[:, 0:1], scalar2=rstd,
                            op0=OP.subtract, op1=OP.mult)
    h = pool.tile([128, DM], F32, name="ln_h")
    nc.vector.tensor_tensor(out=h, in0=xs, in1=g, op=OP.mult)
    nc.vector.tensor_tensor(out=h, in0=h, in1=b, op=OP.add)
    return h


# ---------------------------------------------------------------------------
# host side
# ---------------------------------------------------------------------------

_prog_cache = {}


def _get_program():
    if "nc" not in _prog_cache:
        _prog_cache["nc"] = build_program()
    return _prog_cache["nc"]


def _host_constants():
    f = np.float32
    H = np.array([h for h, w in SHAPES], np.int64)
    W = np.array([w for h, w in SHAPES], np.int64)
    # refdims [(l, xy)]: xy=0 -> W, xy=1 -> H
    refd = np.zeros((LV, 2), f)
    refd[:, 0] = W; refd[:, 1] = H
    refdims = np.broadcast_to(refd.reshape(1, -1), (128, LV * 2)).copy()
    # dims [(m,l,p,xy)] -> W | H (clip max)
    dm = np.zeros((M, LV, P, 2), f)
    dm[:, :, :, 0] = W[None, :, None]
    dm[:, :, :, 1] = H[None, :, None]
    dims = np.broadcast_to(dm.reshape(1, -1), (128, M * LV * P * 2)).copy()
    # dimm1y [(m,l,p)] -> H - 1
    d1 = np.zeros((M, LV, P), f)
    d1[:, :, :] = (H - 1)[None, :, None]
    dimm1y = np.broadcast_to(d1.reshape(1, -1), (128, M * LV * P)).copy()
    # h2t [(m,l,p)] -> H + 2 (column-major x-stride)
    h2 = np.zeros((M, LV, P), f)
    h2[:, :, :] = (H + 2)[None, :, None]
    h2t = np.broadcast_to(h2.reshape(1, -1), (128, M * LV * P)).copy()
    # cidx [(m,l,p)] -> LB_l + (H_l+2) + 1 + 1 (one leading G2 pad row)
    ci = np.zeros((M, LV, P), f)
    for m in range(M):
        for li in range(LV):
            ci[m, li, :] = LB[li] + (H[li] + 2) + 2
    cidx = np.broadcast_to(ci.reshape(1, -1), (128, M * LV * P)).copy()
    return refdims, dims, dimm1y, h2t, cidx


def _build_in_maps(inputs):
    src = np.asarray(inputs["src"], np.float32)
    q_feat = np.asarray(inputs["q_feat"], np.float32)
    q_pos = np.asarray(inputs["q_pos"], np.float32)
    ref = np.asarray(inputs["reference_points"], np.float32)
    ss = np.asarray(inputs["spatial_shapes"])
    lsi_in = np.asarray(inputs["level_start_index"])
    assert src.shape == (B, L, DM), src.shape
    assert [tuple(r) for r in ss.tolist()] == SHAPES, ss
    assert lsi_in.tolist() == [0, 8464, 10580, 11109], lsi_in

    refdims, dims, dimm1y, h2t, cidx = _host_constants()

    def as_bf16(a):
        import ml_dtypes
        return np.asarray(a, np.float32).astype(ml_dtypes.bfloat16)

    common = {
        "Woff": as_bf16(inputs["W_off"]),
        "Wattn": as_bf16(inputs["W_attn"]),
        "Wv": as_bf16(inputs["W_v"]),
        "Wout": as_bf16(inputs["W_out"]),
        "W1": as_bf16(inputs["W1"]),
        "W2": as_bf16(inputs["W2"]),
        "boff": as_bf16(inputs["b_off"]).reshape(1, -1),
        "battn": as_bf16(inputs["b_attn"]).reshape(1, -1),
        "bv": as_bf16(inputs["b_v"]).reshape(1, -1),
        "bout": as_bf16(inputs["b_out"]).reshape(1, -1),
        "b1": as_bf16(inputs["b1"]).reshape(1, -1),
        "b2": as_bf16(inputs["b2"]).reshape(1, -1),
        "ln1g": np.broadcast_to(np.asarray(inputs["ln1_g"], np.float32), (128, DM)).copy(),
        "ln1b": np.broadcast_to(np.asarray(inputs["ln1_b"], np.float32), (128, DM)).copy(),
        "ln2g": np.broadcast_to(np.asarray(inputs["ln2_g"], np.float32), (128, DM)).copy(),
        "ln2b": np.broadcast_to(np.asarray(inputs["ln2_b"], np.float32), (128, DM)).copy(),
        "ident": np.eye(128, dtype=np.float32),
        "ones1": as_bf16(np.ones((1, 128), np.float32)),
        "refdims": refdims, "dims": dims, "dimm1y": dimm1y, "h2t": h2t, "cidx": cidx,
    }

    halves = [(0, LC), (LC, L - LC)]
    in_maps = []
    for core in range(8):
        b = core // 2
        h0, hn = halves[core % 2]
        qf = np.zeros((LCPAD, DM), np.float32)
        qp = np.zeros((LCPAD, DM), np.float32)
        rf = np.zeros((LCPAD, LV, 2), np.float32)
        qf[:hn] = q_feat[b, h0:h0 + hn]
        qp[:hn] = q_pos[b, h0:h0 + hn]
        rf[:hn] = ref[b, h0:h0 + hn]
        m = dict(common)
        m.update({"qf": qf, "qp": qp, "ref": rf, "srcb": src[b]})
        in_maps.append(m)
    return in_maps


def kernel(**inputs):
    from concourse.bass_utils import run_bass_kernel_spmd

    in_maps = _build_in_maps(inputs)
    nc = _get_program()
    res = run_bass_kernel_spmd(nc, in_maps, core_ids=list(range(8)))

    halves = [(0, LC), (LC, L - LC)]
    out = np.zeros((B, L, DM), np.float32)
    for core in range(8):
        b = core // 2
        h0, hn = halves[core % 2]
        out[b, h0:h0 + hn] = res.results[core]["out"][:hn]
    return out
